# revision 51
# baseline (speedup 1.0000x reference)
"""Trainium2 Bass kernel for nn_AdvancedTransformerBlock_15006615733156.

Pre-norm transformer block: LN1 -> QKV -> sliding-window causal attention with
ALiBi (window 256) -> proj residual -> LN2 -> FFN (exact GELU) residual.
B=2, N=2048, D=1024, H=16, HD=64.

Sharding: 8 cores = batch(2) x sequence(4 chunks of 512 rows). The 256-wide
sliding window means each core only needs a 256-row halo of x before its
chunk — no collectives. Chunk-0 cores get a zeroed halo plus a `valid` mask
that zeroes halo V' rows (kills both numerator and softmax denominator).

On-chip layout: scores are computed transposed (S_t[kj, qi]) so the
probability tile is directly consumable as matmul lhsT for PV; the softmax
denominator comes from an appended ones-column in V'. All big matmuls run in
bf16 (1 cyc/row at any N, automatic fast-weight-load on LDWEIGHTS, and far
less PE power than fp32r so the P0 downclock bites less); accumulation stays
fp32 in PSUM, and LN stats / residual adds stay fp32. LN stats use ACT
accum_out.
"""
import sys, math, os
sys.path.insert(0, '/opt/trn_rl_repo')
import numpy as np

B, N, D, H, HD, WIN = 2, 2048, 1024, 16, 64, 256
CHUNK, HALO, ROWS = 512, 256, 768
NEG = -1e30
LN_EPS = 1e-5
NCORES = 8

_cache = {}


def _alibi_slopes(n):
    closest = 2 ** math.floor(math.log2(n))
    base = 2.0 ** (-(2.0 ** (-(math.log2(closest) - 3))))
    return np.power(base, np.arange(1, closest + 1)).astype(np.float32)


def _build_amask():
    """Additive pre-softmax bias, [128, H, 384] bf16 (partition-major so the
    DMA moves one 12 KB contiguous run per partition).

    Softmax over keys j is invariant to any per-query-column constant, so the
    reference's "+1 inside window" and the -slope*i part of the ALiBi term
    drop out; what remains is slope*(j - i) <= 0 inside the band, -1e30
    outside. Values near each column's max are near zero, so bf16's relative
    rounding cannot disturb the softmax weights meaningfully.
    """
    import ml_dtypes
    sl = _alibi_slopes(H)
    j = np.arange(128)[:, None]
    i = np.arange(384)[None, :]
    band = ((i - j) >= 0) & ((i - j) <= 255)
    out = np.where(band[None], sl[:, None, None] * (j - i)[None], NEG)
    return np.ascontiguousarray(
        out.astype(ml_dtypes.bfloat16).transpose(1, 0, 2))


def _kb_span(kb):
    qlo = max(0, kb * 128 - 256)
    qhi = min(512, kb * 128 + 128)
    return qlo, qhi, qlo - (kb * 128 - 256)


_KB_OFF = [0]
for _kb in range(6):
    _q0, _q1, _ = _kb_span(_kb)
    _KB_OFF.append(_KB_OFF[-1] + (_q1 - _q0))  # offsets into the 1536-wide S row


def _build_program(has_qk_bias, has_fc1_bias):
    import concourse.bass as bass
    import concourse.tile as tile
    from concourse import bacc, mybir
    from concourse.masks import make_identity

    F32, F32R = mybir.dt.float32, mybir.dt.float32r
    BF16 = mybir.dt.bfloat16
    # 16-bit P/V' runs PV at 1 cyc/row (fp32 pays 4x at N=65) and gets FWL on
    # the weight load; fp16's 10-bit mantissa keeps softmax-prob rounding at
    # ~5e-4 (bf16's 8-bit costs ~4x more accuracy).
    _pv = os.environ.get("K_PV_DT", "f16")
    PV_DT = {"f32": F32, "bf16": BF16, "f16": mybir.dt.float16}[_pv]
    AF = mybir.ActivationFunctionType
    ADD, MULT = mybir.AluOpType.add, mybir.AluOpType.mult

    nc = bacc.Bacc("TRN2", target_bir_lowering=False, debug=False,
                   num_devices=NCORES)

    xh_d = nc.dram_tensor("xh", [HALO, D], BF16, kind="ExternalInput").ap()
    xl_d = nc.dram_tensor("xl", [CHUNK, D], BF16, kind="ExternalInput").ap()
    wqkv_d = nc.dram_tensor("wqkv", [D, 3 * D], BF16, kind="ExternalInput").ap()
    wproj_d = nc.dram_tensor("wproj", [D, D], BF16, kind="ExternalInput").ap()
    wfc1_d = nc.dram_tensor("wfc1", [D, 4 * D], BF16, kind="ExternalInput").ap()
    wfc2_d = nc.dram_tensor("wfc2", [4 * D, D], BF16, kind="ExternalInput").ap()
    amask_d = nc.dram_tensor("amask", [128, H, 384], BF16, kind="ExternalInput").ap()
    valid_d = nc.dram_tensor("valid", [6, 128], F32, kind="ExternalInput").ap()
    if has_qk_bias:
        qkb_d = nc.dram_tensor("qkbias", [2, 8, 128], F32, kind="ExternalInput").ap()
    if has_fc1_bias:
        b1_d = nc.dram_tensor("b1", [4 * D], F32, kind="ExternalInput").ap()
    y_d = nc.dram_tensor("y", [CHUNK, D], F32, kind="ExternalOutput").ap()

    def ln_block(tc, x_ap, out_ap, small, dump):
        """LayerNorm (no affine) of [128, 1024]: out = (x - mu) * rstd.

        Stats split across engines in parallel: ACT computes E[x^2] via
        Square+accum while DVE reduces E[x]; var = E[x^2] - mu^2.
        """
        sq = small.tile([128, 1], F32, tag="sq", name="sq")
        nc.scalar.activation(dump[:], x_ap, AF.Square, accum_out=sq[:])
        sums = small.tile([128, 1], F32, tag="sums", name="sums")
        nc.vector.tensor_reduce(sums[:], x_ap, mybir.AxisListType.X,
                                mybir.AluOpType.add)
        negmu = small.tile([128, 1], F32, tag="negmu", name="negmu")
        nc.vector.tensor_scalar_mul(negmu[:], sums[:], -1.0 / D)
        m2 = small.tile([128, 1], F32, tag="m2", name="m2")
        nc.vector.tensor_tensor(m2[:], negmu[:], negmu[:], MULT)
        bvar = small.tile([128, 1], F32, tag="bvar", name="bvar")
        nc.vector.tensor_scalar(bvar[:], m2[:], -1.0, LN_EPS, MULT, ADD)
        st = small.tile([128, 1], F32, tag="st", name="st")
        nc.scalar.activation(st[:], sq[:], AF.Sqrt, bias=bvar[:], scale=1.0 / D)
        rstd = small.tile([128, 1], F32, tag="rstd", name="rstd")
        nc.vector.reciprocal(rstd[:], st[:])
        nmr = small.tile([128, 1], F32, tag="nmr", name="nmr")
        nc.vector.tensor_tensor(nmr[:], negmu[:], rstd[:], MULT)
        nc.vector.tensor_scalar(out_ap, x_ap, rstd[:], nmr[:], MULT, ADD)

    with tile.TileContext(nc) as tc:
        # Pool lifetimes form two LIFO stacks (SBUF left/right). Right holds
        # whole-kernel + B..C + F pools; left holds A..B, C..D, E..F1 chains.
        glob = tc.alloc_tile_pool(name="glob", bufs=1, side="right")
        small = tc.alloc_tile_pool(name="small", bufs=8, side="right")
        de = tc.alloc_tile_pool(name="de", bufs=1, side="right")  # x2 (D..end)

        dump = glob.tile([128, D], BF16, name="dump")
        x2_sb = de.tile([128, 4, D], F32, name="x2_sb")
        amask_t = glob.tile([128, H, 384], BF16, name="amask_t")

        # ---------------- Phase A: LN1 + h^T ----------------
        _scA = nc.enter_named_scope('A', False)[0]
        xlp = tc.alloc_tile_pool(name="xlp", bufs=1, side="left")  # x local, A..D
        ab = tc.alloc_tile_pool(name="ab", bufs=1, side="left")
        hT = ab.tile([128, 8, ROWS], BF16, name="hT")

        pa = tc.alloc_tile_pool(name="pa", bufs=2, side="left")
        xh_sb = pa.tile([128, 2, D], BF16, name="xh_sb", bufs=1)
        # halo first (LN block 0 needs it; HWDGE DMAs are FIFO), split so
        # block 0 lands early
        xh_r = xh_d.rearrange("(q p) d -> p q d", p=128)
        nc.sync.dma_start(xh_sb[:, 0], xh_r[:, 0])
        nc.sync.dma_start(xh_sb[:, 1], xh_r[:, 1])
        xl_sb = xlp.tile([128, 4, D], BF16, name="xl_sb")
        xl_r = xl_d.rearrange("(q p) d -> p q d", p=128)
        for q in range(4):
            nc.sync.dma_start(xl_sb[:, q], xl_r[:, q])
        # amask is only needed in phase C; keep it off the SP DGE queue so the
        # first wqkv chunks aren't stuck behind its 1.5 MB.
        nc.scalar.dma_start(amask_t[:], amask_d)
        for blk in range(6):
            x_ap = xh_sb[:, blk] if blk < 2 else xl_sb[:, blk - 2]
            h_pre = pa.tile([128, D], BF16, tag="h_pre", name="h_pre")
            ln_block(tc, x_ap, h_pre[:], small, dump)
            # XBAR DMA transpose: hT[p, kc, blk*128+q] = h_pre[q, kc*128+p].
            # Frees PE (no identity matmuls) and DVE/ACT (no PSUM copies).
            nc.scalar.dma_start_transpose(
                hT[:, :, blk * 128:(blk + 1) * 128], h_pre[:])
        pa.release()

        nc.leave_named_scope('A', _scA, False)
        # ---------------- Phase B: QKV projections ----------------
        _scB = nc.enter_named_scope('B', False)[0]
        bc = tc.alloc_tile_pool(name="bc", bufs=1, side="right")
        QT = bc.tile([128, 8, CHUNK], BF16, name="QT")     # [hd-pair, pair, qi]
        KT = bc.tile([128, 8, ROWS], BF16, name="KT")
        Vp = bc.tile([128, 6, 16 * 65], PV_DT, name="Vp")    # per-head 65-col groups
        valid_t = bc.tile([128, 6], F32, name="valid_t")
        nc.sync.dma_start(valid_t[:], valid_d.rearrange("k p -> p k"))
        if has_qk_bias:
            qkb_t = bc.tile([128, 2, 8], F32, name="qkb_t")
            nc.sync.dma_start(qkb_t[:], qkb_d.rearrange("t g p -> p t g"))

        wb = tc.alloc_tile_pool(name="wb", bufs=2, side="right")
        psb = tc.alloc_tile_pool(name="psb", bufs=2, space="PSUM")

        # Q: 2 groups of 4 head-pairs (512 cols each -> 1 KB DMA runs)
        for g in range(2):
            wq = wb.tile([128, 8, 512], BF16, tag="wqk", name="wq")
            nc.sync.dma_start(
                wq[:], wqkv_d[:, g * 512:(g + 1) * 512]
                .rearrange("(ko p) n -> p ko n", p=128))
            for pp in range(4):
                p = g * 4 + pp
                psq = psb.tile([128, CHUNK], F32, tag="q", name="ps_q", bufs=2)
                for ko in range(8):
                    nc.tensor.matmul(psq[:], wq[:, ko, pp * 128:(pp + 1) * 128],
                                     hT[:, ko, HALO:ROWS],
                                     start=(ko == 0), stop=(ko == 7))
                if has_qk_bias:
                    nc.scalar.activation(QT[:, p], psq[:], AF.Identity,
                                         bias=qkb_t[:, 0, p:p + 1])
                else:
                    nc.scalar.copy(QT[:, p], psq[:])

        # K: 2 groups of 4 head-pairs, N=768
        for g in range(2):
            wk = wb.tile([128, 8, 512], BF16, tag="wqk", name="wk")
            nc.sync.dma_start(
                wk[:], wqkv_d[:, D + g * 512:D + (g + 1) * 512]
                .rearrange("(ko p) n -> p ko n", p=128))
            for pp in range(4):
                p = g * 4 + pp
                psk = psb.tile([128, ROWS], F32, tag="k", name="ps_k", bufs=2)
                for n0, n1 in ((0, 512), (512, 768)):
                    for ko in range(8):
                        nc.tensor.matmul(psk[:, n0:n1],
                                         wk[:, ko, pp * 128:(pp + 1) * 128],
                                         hT[:, ko, n0:n1],
                                         start=(ko == 0), stop=(ko == 7))
                if has_qk_bias:
                    nc.scalar.activation(KT[:, p], psk[:], AF.Identity,
                                         bias=qkb_t[:, 1, p:p + 1])
                else:
                    nc.scalar.copy(KT[:, p], psk[:])

        # V: natural layout [rows, feats], assembled into V' with ones col
        wv = wb.tile([128, 8, D], BF16, tag="wv", name="wv", bufs=1)
        nc.sync.dma_start(wv[:],
                          wqkv_d[:, 2 * D:3 * D].rearrange("(ko p) n -> p ko n", p=128))
        for rb in range(6):
            vp_rb = Vp[:, rb].rearrange("p (h c) -> p h c", c=65)
            for nh in range(2):
                psv = psb.tile([128, 512], F32, tag="v", name="ps_v", bufs=2)
                for ko in range(8):
                    nc.tensor.matmul(psv[:],
                                     hT[:, ko, rb * 128:(rb + 1) * 128],
                                     wv[:, ko, nh * 512:(nh + 1) * 512],
                                     start=(ko == 0), stop=(ko == 7))
                # heads nh*8 .. nh*8+8 of this row-block
                nc.vector.tensor_copy(
                    vp_rb[:, nh * 8:(nh + 1) * 8, 0:64],
                    psv[:].rearrange("p (h c) -> p h c", c=64))
            nc.vector.memset(vp_rb[:, :, 64:65], 1.0)
            nc.vector.tensor_scalar_mul(Vp[:, rb], Vp[:, rb], valid_t[:, rb:rb + 1])

        psb.release()
        wb.release()
        ab.release()  # frees hT

        nc.leave_named_scope('B', _scB, False)
        # ---------------- Phase C: attention ----------------
        _scC = nc.enter_named_scope('C', False)[0]
        cd = tc.alloc_tile_pool(name="cd", bufs=1, side="left")
        O_sb = cd.tile([128, 4, D], BF16, name="O_sb")

        sp = tc.alloc_tile_pool(name="sp", bufs=3, side="right")
        ddw = tc.alloc_tile_pool(name="ddw", bufs=1, side="left")
        psc = tc.alloc_tile_pool(name="psc", bufs=4, space="PSUM")
        pso = tc.alloc_tile_pool(name="pso", bufs=4, space="PSUM")
        wpj = None
        for hp in range(8):
            if hp == 4:
                # prefetch proj weights while attention still runs
                wpj = ddw.tile([128, 8, D], BF16, name="wpj")
                nc.sync.dma_start(wpj[:],
                                  wproj_d.rearrange("(ko p) n -> p ko n", p=128))
            S_pr = sp.tile([128, 2, 1536], F32, tag="S", name="S_pr")
            P_pr = sp.tile([128, 2, 1536], PV_DT, tag="P", name="P_pr")
            for kb in range(6):
                qlo, qhi, il = _kb_span(kb)
                w = qhi - qlo
                # [128, 2, 512]: each head-half starts on a PSUM bank boundary
                pss = psc.tile([128, 2, 512], F32, tag="s", name="ps_s", bufs=2)
                for hh in range(2):
                    pb = hh * 64
                    nc.tensor.matmul(pss[:, hh, :w],
                                     KT[pb:pb + 64, hp, kb * 128:(kb + 1) * 128],
                                     QT[pb:pb + 64, hp, qlo:qhi],
                                     start=True, stop=True)
                nc.vector.tensor_tensor(
                    S_pr[:, :, _KB_OFF[kb]:_KB_OFF[kb] + w],
                    pss[:, :, 0:w],
                    amask_t[:, 2 * hp:2 * hp + 2, il:il + w], ADD)
            # split per head-half so PV of hh=0 can start while hh=1 still exps
            nc.scalar.activation(P_pr[:, 0], S_pr[:, 0], AF.Exp)
            nc.scalar.activation(P_pr[:, 1], S_pr[:, 1], AF.Exp)
            for hh in range(2):
                h_i = hp * 2 + hh
                # all four query-blocks of this head share one PSUM bank so a
                # single reciprocal serves them (16 RECIPROCALs total, not 64)
                po = pso.tile([128, 4, 65], F32, tag="o", name="ps_o")
                for qb in range(4):
                    for t in range(3):
                        kb = qb + t
                        qlo, _, _ = _kb_span(kb)
                        pcol = _KB_OFF[kb] + qb * 128 - qlo
                        nc.tensor.matmul(po[:, qb], P_pr[:, hh, pcol:pcol + 128],
                                         Vp[:, kb, h_i * 65:(h_i + 1) * 65],
                                         start=(t == 0), stop=(t == 2))
                rec = small.tile([128, 4], F32, tag="rec", name="rec")
                nc.vector.reciprocal(rec[:], po[:, :, 64])
                for qb in range(4):
                    if qb % 2 == 0:
                        # balance the divide work across ACT and DVE
                        nc.scalar.activation(
                            O_sb[:, qb, h_i * 64:(h_i + 1) * 64], po[:, qb, 0:64],
                            AF.Copy, scale=rec[:, qb:qb + 1])
                    else:
                        nc.vector.tensor_scalar_mul(
                            O_sb[:, qb, h_i * 64:(h_i + 1) * 64], po[:, qb, 0:64],
                            rec[:, qb:qb + 1])
        pso.release()
        psc.release()
        sp.release()
        bc.release()  # frees QT/KT/Vp

        nc.leave_named_scope('C', _scC, False)
        # ---------------- Phase D: O^T + proj + residual ----------------
        _scD = nc.enter_named_scope('D', False)[0]
        dd = tc.alloc_tile_pool(name="dd", bufs=1, side="left")
        OT = dd.tile([128, 8, CHUNK], BF16, name="OT")
        psd = tc.alloc_tile_pool(name="psd", bufs=2, space="PSUM")
        for qb in range(4):
            nc.scalar.dma_start_transpose(
                OT[:, :, qb * 128:(qb + 1) * 128], O_sb[:, qb])
        for qc in range(4):
            for nh in range(2):
                psp = psd.tile([128, 512], F32, tag="p", name="ps_p")
                for fc in range(8):
                    nc.tensor.matmul(psp[:], OT[:, fc, qc * 128:(qc + 1) * 128],
                                     wpj[:, fc, nh * 512:(nh + 1) * 512],
                                     start=(fc == 0), stop=(fc == 7))
                nc.vector.tensor_tensor(x2_sb[:, qc, nh * 512:(nh + 1) * 512],
                                        psp[:], xl_sb[:, qc, nh * 512:(nh + 1) * 512],
                                        ADD)
        psd.release()
        dd.release()
        ddw.release()
        cd.release()
        xlp.release()

        # ---------------- Phase F pools (right side, before E on left) ------
        ff = tc.alloc_tile_pool(name="ff", bufs=1, side="right")
        ffT = ff.tile([128, 32, CHUNK], BF16, name="ffT")
        y_sb = ff.tile([128, 4, D], F32, name="y_sb")
        # all of wfc2 stays resident (64 KB/partition in bf16): its DMA can
        # then run through E/F1 and fc2 never waits on weights.
        w2all = ff.tile([128, 32, D], BF16, name="w2all")
        w2_r = wfc2_d.rearrange("(fo p) n -> p fo n", p=128)
        if has_fc1_bias:
            b1_t = ff.tile([128, 32], F32, name="b1_t")
            nc.sync.dma_start(b1_t[:], b1_d.rearrange("(fo p) -> p fo", p=128))
        wf = tc.alloc_tile_pool(name="wf", bufs=2, side="right")
        # first half of wfc2 via the SP queue now (it is idle during E)
        for g in range(8):
            nc.sync.dma_start(w2all[:, g * 2:(g + 1) * 2], w2_r[:, g * 2:(g + 1) * 2])

        nc.leave_named_scope('D', _scD, False)
        # ---------------- Phase E: LN2 + h2^T ----------------
        _scE = nc.enter_named_scope('E', False)[0]
        ef = tc.alloc_tile_pool(name="ef", bufs=1, side="left")
        h2T = ef.tile([128, 8, CHUNK], BF16, name="h2T")
        pe_ = tc.alloc_tile_pool(name="pe", bufs=2, side="left")
        for qc in range(4):
            h2_pre = pe_.tile([128, D], BF16, tag="h2_pre", name="h2_pre")
            ln_block(tc, x2_sb[:, qc], h2_pre[:], small, dump)
            nc.scalar.dma_start_transpose(
                h2T[:, :, qc * 128:(qc + 1) * 128], h2_pre[:])
        pe_.release()

        nc.leave_named_scope('E', _scE, False)
        # ---------------- Phase F1: fc1 + GELU ----------------
        _scF1 = nc.enter_named_scope('F1', False)[0]
        psf = tc.alloc_tile_pool(name="psf", bufs=2, space="PSUM")
        for g in range(8):
            w1 = wf.tile([128, 8, 512], BF16, tag="w1", name="w1", bufs=3)
            nc.sync.dma_start(
                w1[:], wfc1_d[:, g * 512:(g + 1) * 512]
                .rearrange("(ko p) n -> p ko n", p=128))
            # trickle the second half of wfc2 in between w1 chunks
            c = 16 + g * 2
            nc.sync.dma_start(w2all[:, c:c + 2], w2_r[:, c:c + 2])
            for f4 in range(4):
                ffc = g * 4 + f4
                psq = psf.tile([128, 512], F32, tag="f", name="ps_f")
                for ko in range(8):
                    nc.tensor.matmul(psq[:], w1[:, ko, f4 * 128:(f4 + 1) * 128],
                                     h2T[:, ko, :], start=(ko == 0), stop=(ko == 7))
                if has_fc1_bias:
                    nc.scalar.activation(ffT[:, ffc, :], psq[:], AF.Gelu,
                                         bias=b1_t[:, ffc:ffc + 1])
                else:
                    nc.scalar.activation(ffT[:, ffc, :], psq[:], AF.Gelu)
        psf.release()
        ef.release()

        nc.leave_named_scope('F1', _scF1, False)
        # ---------------- Phase F2: fc2 + residual + store ----------------
        _scF2 = nc.enter_named_scope('F2', False)[0]
        psy = tc.alloc_tile_pool(name="psy", bufs=4, space="PSUM")
        y_dr = y_d.rearrange("(q p) d -> p q d", p=128)
        # qc-outer: each 128-row output chunk finishes its 32-step
        # accumulation a quarter of the way in, so its residual add and store
        # overlap the remaining compute instead of piling up at the end.
        for qc in range(4):
            ys = [psy.tile([128, 512], F32, tag="y", name=f"ps_y{nh}")
                  for nh in range(2)]
            for ffc in range(32):
                for nh in range(2):
                    nc.tensor.matmul(ys[nh][:],
                                     ffT[:, ffc, qc * 128:(qc + 1) * 128],
                                     w2all[:, ffc, nh * 512:(nh + 1) * 512],
                                     start=(ffc == 0), stop=(ffc == 31))
            for nh in range(2):
                nc.vector.tensor_tensor(y_sb[:, qc, nh * 512:(nh + 1) * 512],
                                        ys[nh][:],
                                        x2_sb[:, qc, nh * 512:(nh + 1) * 512], ADD)
            nc.sync.dma_start(y_dr[:, qc], y_sb[:, qc])
        psy.release()
        wf.release()
        ff.release()
        de.release()
        small.release()
        glob.release()

        nc.leave_named_scope('F2', _scF2, False)

    nc.compile()
    return nc


def kernel(x, qkv_w, qkv_b, proj_w, proj_b, ln1_g, ln1_b, ln2_g, ln2_b,
           fc1_w, fc1_b, fc2_w, fc2_b):
    from concourse.bass_utils import run_bass_kernel_spmd

    x = np.ascontiguousarray(np.asarray(x, dtype=np.float32))
    f32 = lambda a: np.asarray(a, dtype=np.float32)
    qkv_w, qkv_b = f32(qkv_w), f32(qkv_b)
    proj_w, proj_b = f32(proj_w), f32(proj_b)
    fc1_w, fc1_b = f32(fc1_w), f32(fc1_b)
    fc2_w, fc2_b = f32(fc2_w), f32(fc2_b)
    ln1_g, ln1_b = f32(ln1_g), f32(ln1_b)
    ln2_g, ln2_b = f32(ln2_g), f32(ln2_b)

    # Host-side folding: LN affine into the following weight/bias; HD^-0.5 into Wk.
    import ml_dtypes
    bf = ml_dtypes.bfloat16
    scale = HD ** -0.5
    wqkv = ln1_g[:, None] * qkv_w
    bqkv = qkv_b + ln1_b @ qkv_w
    wqkv = np.ascontiguousarray(wqkv)
    wqkv[:, D:2 * D] *= scale
    bqkv = bqkv.copy()
    bqkv[D:2 * D] *= scale
    wfc1 = np.ascontiguousarray(ln2_g[:, None] * fc1_w)
    bfc1 = fc1_b + ln2_b @ fc1_w
    wqkv = np.ascontiguousarray(wqkv.astype(bf))
    wproj16 = np.ascontiguousarray(proj_w.astype(bf))
    wfc1 = np.ascontiguousarray(wfc1.astype(bf))
    wfc216 = np.ascontiguousarray(fc2_w.astype(bf))

    if np.any(bqkv[2 * D:]) or np.any(proj_b) or np.any(fc2_b):
        raise NotImplementedError("nonzero v/proj/fc2 bias not supported")

    has_qk_bias = bool(np.any(bqkv[:2 * D]))
    has_fc1_bias = bool(np.any(bfc1))
    key = (has_qk_bias, has_fc1_bias)
    if key not in _cache:
        _cache[key] = _build_program(*key)
    nc = _cache[key]

    amask = _build_amask()
    in_maps = []
    for c in range(NCORES):
        b, ck = c // 4, c % 4
        g0 = ck * CHUNK
        xl = np.ascontiguousarray(x[b, g0:g0 + CHUNK].astype(bf))
        if ck > 0:
            xhalo = np.ascontiguousarray(x[b, g0 - HALO:g0].astype(bf))
        else:
            xhalo = np.zeros((HALO, D), bf)
        valid = np.ones((6, 128), np.float32)
        if ck == 0:
            valid[:2] = 0.0
        m = {"xh": xhalo, "xl": xl, "wqkv": wqkv, "wproj": wproj16,
             "wfc1": wfc1, "wfc2": wfc216, "amask": amask, "valid": valid}
        if has_qk_bias:
            m["qkbias"] = np.ascontiguousarray(
                bqkv[:2 * D].reshape(2, 8, 128))
        if has_fc1_bias:
            m["b1"] = bfc1
        in_maps.append(m)

    res = run_bass_kernel_spmd(nc, in_maps, core_ids=list(range(NCORES)))
    y = np.empty((B, N, D), np.float32)
    for c in range(NCORES):
        b, ck = c // 4, c % 4
        y[b, ck * CHUNK:(ck + 1) * CHUNK] = res.results[c]["y"]
    return y



# revision 52
# speedup vs baseline: 1.0603x; 1.0603x over previous
"""Trainium2 Bass kernel for nn_AdvancedTransformerBlock_15006615733156.

Pre-norm transformer block: LN1 -> QKV -> sliding-window causal attention with
ALiBi (window 256) -> proj residual -> LN2 -> FFN (exact GELU) residual.
B=2, N=2048, D=1024, H=16, HD=64.

Sharding: 8 cores = batch(2) x sequence(4 chunks of 512 rows). The 256-wide
sliding window means each core only needs a 256-row halo of x before its
chunk — no collectives. Chunk-0 cores get a zeroed halo plus a `valid` mask
that zeroes halo V' rows (kills both numerator and softmax denominator).

On-chip layout: scores are computed transposed (S_t[kj, qi]) so the
probability tile is directly consumable as matmul lhsT for PV; the softmax
denominator comes from an appended ones-column in V'. All big matmuls run in
bf16 (fast-weight-load on LDWEIGHTS, low PE power); accumulation stays fp32
in PSUM, and LN stats / residual adds stay fp32.

Phase structure is chosen to keep the Tensor engine densely fed: the HW
activity manager halves the PE duty limit (k=4/8) within ~10us of the PE
going idle, so PE-sparse phases run their matmuls at half rate. QKV
projections are therefore interleaved per-head-pair with that head-pair's
attention, and LN2 is folded into the proj/residual loop.
"""
import sys, math, os
sys.path.insert(0, '/opt/trn_rl_repo')
import numpy as np

B, N, D, H, HD, WIN = 2, 2048, 1024, 16, 64, 256
CHUNK, HALO, ROWS = 512, 256, 768
NEG = -1e30
LN_EPS = 1e-5
NCORES = 8

_cache = {}


def _alibi_slopes(n):
    closest = 2 ** math.floor(math.log2(n))
    base = 2.0 ** (-(2.0 ** (-(math.log2(closest) - 3))))
    return np.power(base, np.arange(1, closest + 1)).astype(np.float32)


def _build_amask():
    """Additive pre-softmax bias, [128, H, 384] bf16 (partition-major so the
    DMA moves one 12 KB contiguous run per partition).

    Softmax over keys j is invariant to any per-query-column constant, so the
    reference's "+1 inside window" and the -slope*i part of the ALiBi term
    drop out; what remains is slope*(j - i) <= 0 inside the band, -1e30
    outside. Values near each column's max are near zero, so bf16's relative
    rounding cannot disturb the softmax weights meaningfully.
    """
    import ml_dtypes
    sl = _alibi_slopes(H)
    j = np.arange(128)[:, None]
    i = np.arange(384)[None, :]
    band = ((i - j) >= 0) & ((i - j) <= 255)
    out = np.where(band[None], sl[:, None, None] * (j - i)[None], NEG)
    return np.ascontiguousarray(
        out.astype(ml_dtypes.bfloat16).transpose(1, 0, 2))


def _kb_span(kb):
    qlo = max(0, kb * 128 - 256)
    qhi = min(512, kb * 128 + 128)
    return qlo, qhi, qlo - (kb * 128 - 256)


_KB_OFF = [0]
for _kb in range(6):
    _q0, _q1, _ = _kb_span(_kb)
    _KB_OFF.append(_KB_OFF[-1] + (_q1 - _q0))  # offsets into the 1536-wide S row


def _build_program(has_qk_bias, has_fc1_bias):
    import concourse.bass as bass
    import concourse.tile as tile
    from concourse import bacc, mybir
    from concourse.masks import make_identity

    F32 = mybir.dt.float32
    BF16 = mybir.dt.bfloat16
    # 16-bit P/V' runs PV at 1 cyc/row and gets FWL on the weight load; fp16's
    # 10-bit mantissa keeps softmax-prob rounding at ~5e-4.
    _pv = os.environ.get("K_PV_DT", "f16")
    PV_DT = {"f32": F32, "bf16": BF16, "f16": mybir.dt.float16}[_pv]
    AF = mybir.ActivationFunctionType
    ADD, MULT = mybir.AluOpType.add, mybir.AluOpType.mult

    nc = bacc.Bacc("TRN2", target_bir_lowering=False, debug=False,
                   num_devices=NCORES)

    xh_d = nc.dram_tensor("xh", [HALO, D], BF16, kind="ExternalInput").ap()
    xl_d = nc.dram_tensor("xl", [CHUNK, D], BF16, kind="ExternalInput").ap()
    wqkv_d = nc.dram_tensor("wqkv", [D, 3 * D], BF16, kind="ExternalInput").ap()
    wproj_d = nc.dram_tensor("wproj", [D, D], BF16, kind="ExternalInput").ap()
    wfc1_d = nc.dram_tensor("wfc1", [D, 4 * D], BF16, kind="ExternalInput").ap()
    wfc2_d = nc.dram_tensor("wfc2", [4 * D, D], BF16, kind="ExternalInput").ap()
    amask_d = nc.dram_tensor("amask", [128, H, 384], BF16, kind="ExternalInput").ap()
    valid_d = nc.dram_tensor("valid", [6, 128], F32, kind="ExternalInput").ap()
    if has_qk_bias:
        qkb_d = nc.dram_tensor("qkbias", [2, 8, 128], F32, kind="ExternalInput").ap()
    if has_fc1_bias:
        b1_d = nc.dram_tensor("b1", [4 * D], F32, kind="ExternalInput").ap()
    y_d = nc.dram_tensor("y", [CHUNK, D], F32, kind="ExternalOutput").ap()

    def ln_block(tc, x_ap, out_ap, small, dump):
        """LayerNorm (no affine) of [128, 1024]: out = (x - mu) * rstd.

        Stats split across engines in parallel: ACT computes E[x^2] via
        Square+accum while DVE reduces E[x]; var = E[x^2] - mu^2.
        """
        sq = small.tile([128, 1], F32, tag="sq", name="sq")
        nc.scalar.activation(dump[:], x_ap, AF.Square, accum_out=sq[:])
        sums = small.tile([128, 1], F32, tag="sums", name="sums")
        nc.vector.tensor_reduce(sums[:], x_ap, mybir.AxisListType.X,
                                mybir.AluOpType.add)
        negmu = small.tile([128, 1], F32, tag="negmu", name="negmu")
        nc.vector.tensor_scalar_mul(negmu[:], sums[:], -1.0 / D)
        m2 = small.tile([128, 1], F32, tag="m2", name="m2")
        nc.vector.tensor_tensor(m2[:], negmu[:], negmu[:], MULT)
        bvar = small.tile([128, 1], F32, tag="bvar", name="bvar")
        nc.vector.tensor_scalar(bvar[:], m2[:], -1.0, LN_EPS, MULT, ADD)
        st = small.tile([128, 1], F32, tag="st", name="st")
        nc.scalar.activation(st[:], sq[:], AF.Sqrt, bias=bvar[:], scale=1.0 / D)
        rstd = small.tile([128, 1], F32, tag="rstd", name="rstd")
        nc.vector.reciprocal(rstd[:], st[:])
        nmr = small.tile([128, 1], F32, tag="nmr", name="nmr")
        nc.vector.tensor_tensor(nmr[:], negmu[:], rstd[:], MULT)
        nc.vector.tensor_scalar(out_ap, x_ap, rstd[:], nmr[:], MULT, ADD)

    with tile.TileContext(nc) as tc:
        # Pool lifetimes form two LIFO stacks (SBUF left/right).
        glob = tc.alloc_tile_pool(name="glob", bufs=1, side="right")
        small = tc.alloc_tile_pool(name="small", bufs=8, side="right")
        de = tc.alloc_tile_pool(name="de", bufs=1, side="right")  # x2 (D..end)

        ident = glob.tile([128, 128], BF16, name="ident")
        make_identity(nc, ident[:])
        dump = glob.tile([128, D], BF16, name="dump")
        x2_sb = de.tile([128, 4, D], F32, name="x2_sb")

        # bc: everything attention-scoped (released at end of attention)
        bc = tc.alloc_tile_pool(name="bc", bufs=1, side="right")
        hT = bc.tile([128, 8, ROWS], BF16, name="hT")
        QT = bc.tile([128, 8, CHUNK], BF16, name="QT")     # [hd-pair, pair, qi]
        KT = bc.tile([128, 8, ROWS], BF16, name="KT")
        Vp = bc.tile([128, 6, 16 * 65], PV_DT, name="Vp")  # per-head 65-col groups
        amask_t = bc.tile([128, H, 384], BF16, name="amask_t")
        valid_t = bc.tile([128, 6], F32, name="valid_t")

        # ---------------- Phase A: LN1 + h^T ----------------
        _scA = nc.enter_named_scope('A', False)[0]
        xlp = tc.alloc_tile_pool(name="xlp", bufs=1, side="left")  # x local, A..D
        pa = tc.alloc_tile_pool(name="pa", bufs=2, side="left")
        psa = tc.alloc_tile_pool(name="psa", bufs=2, space="PSUM")
        xh_sb = pa.tile([128, 2, D], BF16, name="xh_sb", bufs=1)
        # halo first (LN block 0 needs it; HWDGE DMAs are FIFO), split so
        # block 0 lands early
        xh_r = xh_d.rearrange("(q p) d -> p q d", p=128)
        nc.sync.dma_start(xh_sb[:, 0], xh_r[:, 0])
        nc.sync.dma_start(xh_sb[:, 1], xh_r[:, 1])
        xl_sb = xlp.tile([128, 4, D], BF16, name="xl_sb")
        xl_r = xl_d.rearrange("(q p) d -> p q d", p=128)
        for q in range(4):
            nc.sync.dma_start(xl_sb[:, q], xl_r[:, q])
        nc.sync.dma_start(valid_t[:], valid_d.rearrange("k p -> p k"))
        # amask is only needed by the softmax; keep it off the SP DGE queue so
        # the first weight chunks aren't stuck behind its 1.5 MB.
        nc.scalar.dma_start(amask_t[:], amask_d)
        if has_qk_bias:
            qkb_t = bc.tile([128, 2, 8], F32, name="qkb_t")
            nc.scalar.dma_start(qkb_t[:], qkb_d.rearrange("t g p -> p t g"))
        for blk in range(6):
            x_ap = xh_sb[:, blk] if blk < 2 else xl_sb[:, blk - 2]
            h_pre = pa.tile([128, D], BF16, tag="h_pre", name="h_pre")
            ln_block(tc, x_ap, h_pre[:], small, dump)
            for kc in range(8):
                pst = psa.tile([128, 128], BF16, tag="tr", name="ps_tr")
                nc.tensor.transpose(pst[:], h_pre[:, kc * 128:(kc + 1) * 128],
                                    ident[:])
                if kc % 2 == 0:
                    nc.vector.tensor_copy(hT[:, kc, blk * 128:(blk + 1) * 128], pst[:])
                else:
                    nc.scalar.copy(hT[:, kc, blk * 128:(blk + 1) * 128], pst[:])
        psa.release()
        pa.release()

        nc.leave_named_scope('A', _scA, False)
        # ---------------- Phase B+C: V, then per-head-pair QK + attention ----
        # Interleaving QK projections with attention keeps the PE dense, so
        # the activity manager never drops it to half duty mid-attention.
        _scB = nc.enter_named_scope('B', False)[0]
        wb = tc.alloc_tile_pool(name="wb", bufs=2, side="right")
        psb = tc.alloc_tile_pool(name="psb", bufs=2, space="PSUM")

        # V first: its compute covers the Q/K weight DMAs
        wv = wb.tile([128, 8, D], BF16, tag="wv", name="wv", bufs=1)
        nc.sync.dma_start(wv[:],
                          wqkv_d[:, 2 * D:3 * D].rearrange("(ko p) n -> p ko n", p=128))
        for rb in range(6):
            vp_rb = Vp[:, rb].rearrange("p (h c) -> p h c", c=65)
            for nh in range(2):
                psv = psb.tile([128, 512], F32, tag="v", name="ps_v", bufs=2)
                for ko in range(8):
                    nc.tensor.matmul(psv[:],
                                     hT[:, ko, rb * 128:(rb + 1) * 128],
                                     wv[:, ko, nh * 512:(nh + 1) * 512],
                                     start=(ko == 0), stop=(ko == 7))
                # heads nh*8 .. nh*8+8 of this row-block
                nc.vector.tensor_copy(
                    vp_rb[:, nh * 8:(nh + 1) * 8, 0:64],
                    psv[:].rearrange("p (h c) -> p h c", c=64))
            nc.vector.memset(vp_rb[:, :, 64:65], 1.0)
            nc.vector.tensor_scalar_mul(Vp[:, rb], Vp[:, rb], valid_t[:, rb:rb + 1])
        psb.release()

        nc.leave_named_scope('B', _scB, False)
        _scC = nc.enter_named_scope('C', False)[0]
        cd = tc.alloc_tile_pool(name="cd", bufs=1, side="left")
        ddw = tc.alloc_tile_pool(name="ddw", bufs=1, side="left")
        O_sb = cd.tile([128, 4, D], BF16, name="O_sb")

        sp = tc.alloc_tile_pool(name="sp", bufs=2, side="right")
        psq_p = tc.alloc_tile_pool(name="psq", bufs=1, space="PSUM")
        psk_p = tc.alloc_tile_pool(name="psk", bufs=2, space="PSUM")
        psc = tc.alloc_tile_pool(name="psc", bufs=2, space="PSUM")
        pso = tc.alloc_tile_pool(name="pso", bufs=1, space="PSUM")

        wpj = None
        for g in range(2):
            wq = wb.tile([128, 8, 512], BF16, tag="wq", name="wq", bufs=2)
            nc.sync.dma_start(
                wq[:], wqkv_d[:, g * 512:(g + 1) * 512]
                .rearrange("(ko p) n -> p ko n", p=128))
            wk = wb.tile([128, 8, 512], BF16, tag="wk", name="wk", bufs=2)
            nc.sync.dma_start(
                wk[:], wqkv_d[:, D + g * 512:D + (g + 1) * 512]
                .rearrange("(ko p) n -> p ko n", p=128))
            for pp in range(4):
                hp = g * 4 + pp
                if hp == 4:
                    # prefetch proj weights while attention still runs
                    wpj = ddw.tile([128, 8, D], BF16, name="wpj")
                    nc.sync.dma_start(wpj[:],
                                      wproj_d.rearrange("(ko p) n -> p ko n", p=128))
                # Q for head-pair hp
                psq = psq_p.tile([128, CHUNK], F32, tag="q", name="ps_q")
                for ko in range(8):
                    nc.tensor.matmul(psq[:], wq[:, ko, pp * 128:(pp + 1) * 128],
                                     hT[:, ko, HALO:ROWS],
                                     start=(ko == 0), stop=(ko == 7))
                if has_qk_bias:
                    nc.scalar.activation(QT[:, hp], psq[:], AF.Identity,
                                         bias=qkb_t[:, 0, hp:hp + 1])
                else:
                    nc.scalar.copy(QT[:, hp], psq[:])
                # K for head-pair hp (two 384-wide chains)
                for n0 in (0, 384):
                    psk = psk_p.tile([128, 384], F32, tag="k", name="ps_k")
                    for ko in range(8):
                        nc.tensor.matmul(psk[:],
                                         wk[:, ko, pp * 128:(pp + 1) * 128],
                                         hT[:, ko, n0:n0 + 384],
                                         start=(ko == 0), stop=(ko == 7))
                    if has_qk_bias:
                        nc.scalar.activation(KT[:, hp, n0:n0 + 384], psk[:],
                                             AF.Identity,
                                             bias=qkb_t[:, 1, hp:hp + 1])
                    else:
                        nc.scalar.copy(KT[:, hp, n0:n0 + 384], psk[:])

                # attention for this head-pair
                S_pr = sp.tile([128, 2, 1536], F32, tag="S", name="S_pr")
                P_pr = sp.tile([128, 2, 1536], PV_DT, tag="P", name="P_pr")
                for kb in range(6):
                    qlo, qhi, il = _kb_span(kb)
                    w = qhi - qlo
                    # [128, 2, 512]: head-halves on PSUM bank boundaries
                    pss = psc.tile([128, 2, 512], F32, tag="s", name="ps_s")
                    for hh in range(2):
                        pb = hh * 64
                        nc.tensor.matmul(pss[:, hh, :w],
                                         KT[pb:pb + 64, hp, kb * 128:(kb + 1) * 128],
                                         QT[pb:pb + 64, hp, qlo:qhi],
                                         start=True, stop=True)
                    nc.vector.tensor_tensor(
                        S_pr[:, :, _KB_OFF[kb]:_KB_OFF[kb] + w],
                        pss[:, :, 0:w],
                        amask_t[:, 2 * hp:2 * hp + 2, il:il + w], ADD)
                # split per head-half so PV of hh=0 starts while hh=1 exps
                nc.scalar.activation(P_pr[:, 0], S_pr[:, 0], AF.Exp)
                nc.scalar.activation(P_pr[:, 1], S_pr[:, 1], AF.Exp)
                for hh in range(2):
                    h_i = hp * 2 + hh
                    # four query-blocks share one PSUM bank: one reciprocal
                    po = pso.tile([128, 4, 65], F32, tag="o", name="ps_o")
                    for qb in range(4):
                        for t in range(3):
                            kb = qb + t
                            qlo, _, _ = _kb_span(kb)
                            pcol = _KB_OFF[kb] + qb * 128 - qlo
                            nc.tensor.matmul(po[:, qb], P_pr[:, hh, pcol:pcol + 128],
                                             Vp[:, kb, h_i * 65:(h_i + 1) * 65],
                                             start=(t == 0), stop=(t == 2))
                    rec = small.tile([128, 4], F32, tag="rec", name="rec")
                    nc.vector.reciprocal(rec[:], po[:, :, 64])
                    for qb in range(4):
                        if qb % 2 == 0:
                            # balance the divide work across ACT and DVE
                            nc.scalar.activation(
                                O_sb[:, qb, h_i * 64:(h_i + 1) * 64],
                                po[:, qb, 0:64], AF.Copy, scale=rec[:, qb:qb + 1])
                        else:
                            nc.vector.tensor_scalar_mul(
                                O_sb[:, qb, h_i * 64:(h_i + 1) * 64],
                                po[:, qb, 0:64], rec[:, qb:qb + 1])
        pso.release()
        psc.release()
        psk_p.release()
        psq_p.release()
        sp.release()
        wb.release()
        bc.release()  # frees hT/QT/KT/Vp/amask

        nc.leave_named_scope('C', _scC, False)
        # ---------------- Phase F pools (right side) ------------------------
        ff = tc.alloc_tile_pool(name="ff", bufs=1, side="right")
        ffT = ff.tile([128, 32, CHUNK], BF16, name="ffT")
        y_sb = ff.tile([128, 4, D], F32, name="y_sb")
        h2T = ff.tile([128, 8, CHUNK], BF16, name="h2T")
        # all of wfc2 stays resident (64 KB/partition in bf16): its DMA runs
        # through D/F1 and fc2 never waits on weights.
        w2all = ff.tile([128, 32, D], BF16, name="w2all")
        w2_r = wfc2_d.rearrange("(fo p) n -> p fo n", p=128)
        if has_fc1_bias:
            b1_t = ff.tile([128, 32], F32, name="b1_t")
            nc.sync.dma_start(b1_t[:], b1_d.rearrange("(fo p) -> p fo", p=128))
        wf = tc.alloc_tile_pool(name="wf", bufs=2, side="right")
        for g in range(2):
            nc.sync.dma_start(w2all[:, g * 2:(g + 1) * 2], w2_r[:, g * 2:(g + 1) * 2])

        # ------- Phase D: O^T + proj + residual + LN2 + h2^T, per qc --------
        _scD = nc.enter_named_scope('D', False)[0]
        dd = tc.alloc_tile_pool(name="dd", bufs=1, side="left")
        OT = dd.tile([128, 8, CHUNK], BF16, name="OT")
        pe_ = tc.alloc_tile_pool(name="pe", bufs=2, side="left")
        psd = tc.alloc_tile_pool(name="psd", bufs=2, space="PSUM")
        for qc in range(4):
            for fc in range(8):
                pst = psd.tile([128, 128], BF16, tag="tr", name="ps_tr2")
                nc.tensor.transpose(pst[:], O_sb[:, qc, fc * 128:(fc + 1) * 128],
                                    ident[:])
                if fc % 2 == 0:
                    nc.vector.tensor_copy(OT[:, fc, qc * 128:(qc + 1) * 128], pst[:])
                else:
                    nc.scalar.copy(OT[:, fc, qc * 128:(qc + 1) * 128], pst[:])
            for nh in range(2):
                psp = psd.tile([128, 512], F32, tag="p", name="ps_p")
                for fc in range(8):
                    nc.tensor.matmul(psp[:], OT[:, fc, qc * 128:(qc + 1) * 128],
                                     wpj[:, fc, nh * 512:(nh + 1) * 512],
                                     start=(fc == 0), stop=(fc == 7))
                nc.vector.tensor_tensor(x2_sb[:, qc, nh * 512:(nh + 1) * 512],
                                        psp[:], xl_sb[:, qc, nh * 512:(nh + 1) * 512],
                                        ADD)
            # LN2 + h2^T for this chunk, immediately (keeps PE dense: its
            # transposes overlap the next chunk's proj)
            h2_pre = pe_.tile([128, D], BF16, tag="h2_pre", name="h2_pre")
            ln_block(tc, x2_sb[:, qc], h2_pre[:], small, dump)
            for kc in range(8):
                pst = psd.tile([128, 128], BF16, tag="tr", name="ps_tr2")
                nc.tensor.transpose(pst[:], h2_pre[:, kc * 128:(kc + 1) * 128],
                                    ident[:])
                if kc % 2 == 0:
                    nc.vector.tensor_copy(h2T[:, kc, qc * 128:(qc + 1) * 128], pst[:])
                else:
                    nc.scalar.copy(h2T[:, kc, qc * 128:(qc + 1) * 128], pst[:])
        psd.release()
        pe_.release()
        dd.release()
        ddw.release()
        cd.release()
        xlp.release()

        nc.leave_named_scope('D', _scD, False)
        # ---------------- Phase F1: fc1 + GELU ----------------
        _scF1 = nc.enter_named_scope('F1', False)[0]
        psf = tc.alloc_tile_pool(name="psf", bufs=2, space="PSUM")
        for g in range(8):
            w1 = wf.tile([128, 8, 512], BF16, tag="w1", name="w1", bufs=2)
            nc.sync.dma_start(
                w1[:], wfc1_d[:, g * 512:(g + 1) * 512]
                .rearrange("(ko p) n -> p ko n", p=128))
            # trickle the rest of wfc2 between w1 chunks (4+28 = all 32)
            c = 4 + g * 3
            nw = 3 if g < 7 else 7
            nc.sync.dma_start(w2all[:, c:c + nw], w2_r[:, c:c + nw])
            for f4 in range(4):
                ffc = g * 4 + f4
                psq = psf.tile([128, 512], F32, tag="f", name="ps_f")
                for ko in range(8):
                    nc.tensor.matmul(psq[:], w1[:, ko, f4 * 128:(f4 + 1) * 128],
                                     h2T[:, ko, :], start=(ko == 0), stop=(ko == 7))
                if has_fc1_bias:
                    nc.scalar.activation(ffT[:, ffc, :], psq[:], AF.Gelu,
                                         bias=b1_t[:, ffc:ffc + 1])
                else:
                    nc.scalar.activation(ffT[:, ffc, :], psq[:], AF.Gelu)
        psf.release()

        nc.leave_named_scope('F1', _scF1, False)
        # ---------------- Phase F2: fc2 + residual + store ----------------
        _scF2 = nc.enter_named_scope('F2', False)[0]
        psy = tc.alloc_tile_pool(name="psy", bufs=4, space="PSUM")
        y_dr = y_d.rearrange("(q p) d -> p q d", p=128)
        # qc-outer: each 128-row output chunk finishes its accumulation a
        # quarter of the way in, so its residual add and store overlap the
        # remaining compute instead of piling up at the end.
        for qc in range(4):
            ys = [psy.tile([128, 512], F32, tag="y", name=f"ps_y{nh}")
                  for nh in range(2)]
            for ffc in range(32):
                for nh in range(2):
                    nc.tensor.matmul(ys[nh][:],
                                     ffT[:, ffc, qc * 128:(qc + 1) * 128],
                                     w2all[:, ffc, nh * 512:(nh + 1) * 512],
                                     start=(ffc == 0), stop=(ffc == 31))
            for nh in range(2):
                nc.vector.tensor_tensor(y_sb[:, qc, nh * 512:(nh + 1) * 512],
                                        ys[nh][:],
                                        x2_sb[:, qc, nh * 512:(nh + 1) * 512], ADD)
            nc.sync.dma_start(y_dr[:, qc], y_sb[:, qc])
        psy.release()
        wf.release()
        ff.release()
        de.release()
        small.release()
        glob.release()

        nc.leave_named_scope('F2', _scF2, False)

    nc.compile()
    return nc


def kernel(x, qkv_w, qkv_b, proj_w, proj_b, ln1_g, ln1_b, ln2_g, ln2_b,
           fc1_w, fc1_b, fc2_w, fc2_b):
    from concourse.bass_utils import run_bass_kernel_spmd

    x = np.ascontiguousarray(np.asarray(x, dtype=np.float32))
    f32 = lambda a: np.asarray(a, dtype=np.float32)
    qkv_w, qkv_b = f32(qkv_w), f32(qkv_b)
    proj_w, proj_b = f32(proj_w), f32(proj_b)
    fc1_w, fc1_b = f32(fc1_w), f32(fc1_b)
    fc2_w, fc2_b = f32(fc2_w), f32(fc2_b)
    ln1_g, ln1_b = f32(ln1_g), f32(ln1_b)
    ln2_g, ln2_b = f32(ln2_g), f32(ln2_b)

    # Host-side folding: LN affine into the following weight/bias; HD^-0.5 into Wk.
    import ml_dtypes
    bf = ml_dtypes.bfloat16
    scale = HD ** -0.5
    wqkv = ln1_g[:, None] * qkv_w
    bqkv = qkv_b + ln1_b @ qkv_w
    wqkv = np.ascontiguousarray(wqkv)
    wqkv[:, D:2 * D] *= scale
    bqkv = bqkv.copy()
    bqkv[D:2 * D] *= scale
    wfc1 = np.ascontiguousarray(ln2_g[:, None] * fc1_w)
    bfc1 = fc1_b + ln2_b @ fc1_w
    wqkv = np.ascontiguousarray(wqkv.astype(bf))
    wproj16 = np.ascontiguousarray(proj_w.astype(bf))
    wfc1 = np.ascontiguousarray(wfc1.astype(bf))
    wfc216 = np.ascontiguousarray(fc2_w.astype(bf))

    if np.any(bqkv[2 * D:]) or np.any(proj_b) or np.any(fc2_b):
        raise NotImplementedError("nonzero v/proj/fc2 bias not supported")

    has_qk_bias = bool(np.any(bqkv[:2 * D]))
    has_fc1_bias = bool(np.any(bfc1))
    key = (has_qk_bias, has_fc1_bias)
    if key not in _cache:
        _cache[key] = _build_program(*key)
    nc = _cache[key]

    amask = _build_amask()
    in_maps = []
    for c in range(NCORES):
        b, ck = c // 4, c % 4
        g0 = ck * CHUNK
        xl = np.ascontiguousarray(x[b, g0:g0 + CHUNK].astype(bf))
        if ck > 0:
            xhalo = np.ascontiguousarray(x[b, g0 - HALO:g0].astype(bf))
        else:
            xhalo = np.zeros((HALO, D), bf)
        valid = np.ones((6, 128), np.float32)
        if ck == 0:
            valid[:2] = 0.0
        m = {"xh": xhalo, "xl": xl, "wqkv": wqkv, "wproj": wproj16,
             "wfc1": wfc1, "wfc2": wfc216, "amask": amask, "valid": valid}
        if has_qk_bias:
            m["qkbias"] = np.ascontiguousarray(
                bqkv[:2 * D].reshape(2, 8, 128))
        if has_fc1_bias:
            m["b1"] = bfc1
        in_maps.append(m)

    res = run_bass_kernel_spmd(nc, in_maps, core_ids=list(range(NCORES)))
    y = np.empty((B, N, D), np.float32)
    for c in range(NCORES):
        b, ck = c // 4, c % 4
        y[b, ck * CHUNK:(ck + 1) * CHUNK] = res.results[c]["y"]
    return y


# revision 53
# speedup vs baseline: 1.1183x; 1.0546x over previous
"""Trainium2 Bass kernel for nn_AdvancedTransformerBlock_15006615733156.

Pre-norm transformer block: LN1 -> QKV -> sliding-window causal attention with
ALiBi (window 256) -> proj residual -> LN2 -> FFN (exact GELU) residual.
B=2, N=2048, D=1024, H=16, HD=64.

Sharding: 8 cores = batch(2) x sequence(4 chunks of 512 rows). The 256-wide
sliding window means each core only needs a 256-row halo of x before its
chunk — no collectives. Chunk-0 cores get a zeroed halo plus a `valid` mask
that zeroes halo V' rows (kills both numerator and softmax denominator).

On-chip layout: scores are computed transposed (S_t[kj, qi]) so the
probability tile is directly consumable as matmul lhsT for PV; the softmax
denominator comes from an appended ones-column in V'. All big matmuls run in
bf16 (fast-weight-load on LDWEIGHTS, low PE power); accumulation stays fp32
in PSUM, and LN stats / residual adds stay fp32.

Phase structure is chosen to keep the Tensor engine densely fed: the HW
activity manager halves the PE duty limit (k=4/8) within ~10us of the PE
going idle, so PE-sparse phases run their matmuls at half rate. QKV
projections are therefore interleaved per-head-pair with that head-pair's
attention, and LN2 is folded into the proj/residual loop.
"""
import sys, math, os
sys.path.insert(0, '/opt/trn_rl_repo')
import numpy as np

B, N, D, H, HD, WIN = 2, 2048, 1024, 16, 64, 256
CHUNK, HALO, ROWS = 512, 256, 768
NEG = -1e30
LN_EPS = 1e-5
NCORES = 8

_cache = {}


def _alibi_slopes(n):
    closest = 2 ** math.floor(math.log2(n))
    base = 2.0 ** (-(2.0 ** (-(math.log2(closest) - 3))))
    return np.power(base, np.arange(1, closest + 1)).astype(np.float32)


def _build_amask():
    """Additive pre-softmax bias, [128, H, 384] bf16 (partition-major so the
    DMA moves one 12 KB contiguous run per partition).

    Softmax over keys j is invariant to any per-query-column constant, so the
    reference's "+1 inside window" and the -slope*i part of the ALiBi term
    drop out; what remains is slope*(j - i) <= 0 inside the band, -1e30
    outside. Values near each column's max are near zero, so bf16's relative
    rounding cannot disturb the softmax weights meaningfully.
    """
    import ml_dtypes
    sl = _alibi_slopes(H)
    j = np.arange(128)[:, None]
    i = np.arange(384)[None, :]
    band = ((i - j) >= 0) & ((i - j) <= 255)
    out = np.where(band[None], sl[:, None, None] * (j - i)[None], NEG)
    return np.ascontiguousarray(
        out.astype(ml_dtypes.bfloat16).transpose(1, 0, 2))


def _kb_span(kb):
    qlo = max(0, kb * 128 - 256)
    qhi = min(512, kb * 128 + 128)
    return qlo, qhi, qlo - (kb * 128 - 256)


_KB_OFF = [0]
for _kb in range(6):
    _q0, _q1, _ = _kb_span(_kb)
    _KB_OFF.append(_KB_OFF[-1] + (_q1 - _q0))  # offsets into the 1536-wide S row


def _build_program(has_qk_bias, has_fc1_bias):
    import concourse.bass as bass
    import concourse.tile as tile
    from concourse import bacc, mybir
    from concourse.masks import make_identity

    F32 = mybir.dt.float32
    BF16 = mybir.dt.bfloat16
    # 16-bit P/V' runs PV at 1 cyc/row and gets FWL on the weight load; fp16's
    # 10-bit mantissa keeps softmax-prob rounding at ~5e-4.
    _pv = os.environ.get("K_PV_DT", "f16")
    PV_DT = {"f32": F32, "bf16": BF16, "f16": mybir.dt.float16}[_pv]
    AF = mybir.ActivationFunctionType
    ADD, MULT = mybir.AluOpType.add, mybir.AluOpType.mult

    nc = bacc.Bacc("TRN2", target_bir_lowering=False, debug=False,
                   num_devices=NCORES)

    xh_d = nc.dram_tensor("xh", [HALO, D], BF16, kind="ExternalInput").ap()
    xl_d = nc.dram_tensor("xl", [CHUNK, D], BF16, kind="ExternalInput").ap()
    wqkv_d = nc.dram_tensor("wqkv", [D, 3 * D], BF16, kind="ExternalInput").ap()
    wproj_d = nc.dram_tensor("wproj", [D, D], BF16, kind="ExternalInput").ap()
    wfc1_d = nc.dram_tensor("wfc1", [D, 4 * D], BF16, kind="ExternalInput").ap()
    wfc2_d = nc.dram_tensor("wfc2", [4 * D, D], BF16, kind="ExternalInput").ap()
    amask_d = nc.dram_tensor("amask", [128, H, 384], BF16, kind="ExternalInput").ap()
    valid_d = nc.dram_tensor("valid", [6, 128], F32, kind="ExternalInput").ap()
    if has_qk_bias:
        qkb_d = nc.dram_tensor("qkbias", [2, 8, 128], F32, kind="ExternalInput").ap()
    if has_fc1_bias:
        b1_d = nc.dram_tensor("b1", [4 * D], F32, kind="ExternalInput").ap()
    y_d = nc.dram_tensor("y", [CHUNK, D], F32, kind="ExternalOutput").ap()

    def ln_block(tc, x_ap, out_ap, small, dump):
        """LayerNorm (no affine) of [128, 1024]: out = (x - mu) * rstd.

        Stats split across engines in parallel: ACT computes E[x^2] via
        Square+accum while DVE reduces E[x]; var = E[x^2] - mu^2.
        """
        sq = small.tile([128, 1], F32, tag="sq", name="sq")
        nc.scalar.activation(dump[:], x_ap, AF.Square, accum_out=sq[:])
        sums = small.tile([128, 1], F32, tag="sums", name="sums")
        nc.vector.tensor_reduce(sums[:], x_ap, mybir.AxisListType.X,
                                mybir.AluOpType.add)
        negmu = small.tile([128, 1], F32, tag="negmu", name="negmu")
        nc.vector.tensor_scalar_mul(negmu[:], sums[:], -1.0 / D)
        m2 = small.tile([128, 1], F32, tag="m2", name="m2")
        nc.vector.tensor_tensor(m2[:], negmu[:], negmu[:], MULT)
        bvar = small.tile([128, 1], F32, tag="bvar", name="bvar")
        nc.vector.tensor_scalar(bvar[:], m2[:], -1.0, LN_EPS, MULT, ADD)
        st = small.tile([128, 1], F32, tag="st", name="st")
        nc.scalar.activation(st[:], sq[:], AF.Sqrt, bias=bvar[:], scale=1.0 / D)
        rstd = small.tile([128, 1], F32, tag="rstd", name="rstd")
        nc.vector.reciprocal(rstd[:], st[:])
        nmr = small.tile([128, 1], F32, tag="nmr", name="nmr")
        nc.vector.tensor_tensor(nmr[:], negmu[:], rstd[:], MULT)
        nc.vector.tensor_scalar(out_ap, x_ap, rstd[:], nmr[:], MULT, ADD)

    with tile.TileContext(nc) as tc:
        # Pool lifetimes form two LIFO stacks (SBUF left/right).
        glob = tc.alloc_tile_pool(name="glob", bufs=1, side="right")
        small = tc.alloc_tile_pool(name="small", bufs=8, side="right")
        de = tc.alloc_tile_pool(name="de", bufs=1, side="right")  # x2 (D..end)

        ident = glob.tile([128, 128], BF16, name="ident")
        make_identity(nc, ident[:])
        dump = glob.tile([128, D], BF16, name="dump")
        x2_sb = de.tile([128, 4, D], F32, name="x2_sb")

        # bc: everything attention-scoped (released at end of attention)
        bc = tc.alloc_tile_pool(name="bc", bufs=1, side="right")
        hT = bc.tile([128, 8, ROWS], BF16, name="hT")
        QT = bc.tile([128, 8, CHUNK], BF16, name="QT")     # [hd-pair, pair, qi]
        KT = bc.tile([128, 8, ROWS], BF16, name="KT")
        Vp = bc.tile([128, 6, 16 * 65], PV_DT, name="Vp")  # per-head 65-col groups
        amask_t = bc.tile([128, H, 384], BF16, name="amask_t")
        valid_t = bc.tile([128, 6], F32, name="valid_t")

        # ---------------- Phase A: LN1 + h^T ----------------
        _scA = nc.enter_named_scope('A', False)[0]
        xlp = tc.alloc_tile_pool(name="xlp", bufs=1, side="left")  # x local, A..D
        pa = tc.alloc_tile_pool(name="pa", bufs=2, side="left")
        psa = tc.alloc_tile_pool(name="psa", bufs=2, space="PSUM")
        xh_sb = pa.tile([128, 2, D], BF16, name="xh_sb", bufs=1)
        # halo first (LN block 0 needs it; HWDGE DMAs are FIFO), split so
        # block 0 lands early
        xh_r = xh_d.rearrange("(q p) d -> p q d", p=128)
        nc.sync.dma_start(xh_sb[:, 0], xh_r[:, 0])
        nc.sync.dma_start(xh_sb[:, 1], xh_r[:, 1])
        xl_sb = xlp.tile([128, 4, D], BF16, name="xl_sb")
        xl_r = xl_d.rearrange("(q p) d -> p q d", p=128)
        for q in range(4):
            nc.sync.dma_start(xl_sb[:, q], xl_r[:, q])
        nc.sync.dma_start(valid_t[:], valid_d.rearrange("k p -> p k"))
        # amask is only needed by the softmax; keep it off the SP DGE queue so
        # the first weight chunks aren't stuck behind its 1.5 MB.
        nc.scalar.dma_start(amask_t[:], amask_d)
        if has_qk_bias:
            qkb_t = bc.tile([128, 2, 8], F32, name="qkb_t")
            nc.scalar.dma_start(qkb_t[:], qkb_d.rearrange("t g p -> p t g"))
        for blk in range(6):
            x_ap = xh_sb[:, blk] if blk < 2 else xl_sb[:, blk - 2]
            h_pre = pa.tile([128, D], BF16, tag="h_pre", name="h_pre")
            ln_block(tc, x_ap, h_pre[:], small, dump)
            for kc in range(8):
                pst = psa.tile([128, 128], BF16, tag="tr", name="ps_tr")
                nc.tensor.transpose(pst[:], h_pre[:, kc * 128:(kc + 1) * 128],
                                    ident[:])
                if kc % 2 == 0:
                    nc.vector.tensor_copy(hT[:, kc, blk * 128:(blk + 1) * 128], pst[:])
                else:
                    nc.scalar.copy(hT[:, kc, blk * 128:(blk + 1) * 128], pst[:])
        psa.release()
        pa.release()

        nc.leave_named_scope('A', _scA, False)
        # ---------------- Phase B+C: V, then per-head-pair QK + attention ----
        # Interleaving QK projections with attention keeps the PE dense, so
        # the activity manager never drops it to half duty mid-attention.
        _scB = nc.enter_named_scope('B', False)[0]
        wb = tc.alloc_tile_pool(name="wb", bufs=2, side="right")
        psb = tc.alloc_tile_pool(name="psb", bufs=2, space="PSUM")

        # V first: its compute covers the Q/K weight DMAs
        wv = wb.tile([128, 8, D], BF16, tag="wv", name="wv", bufs=1)
        nc.sync.dma_start(wv[:],
                          wqkv_d[:, 2 * D:3 * D].rearrange("(ko p) n -> p ko n", p=128))
        for rb in range(6):
            vp_rb = Vp[:, rb].rearrange("p (h c) -> p h c", c=65)
            for nh in range(2):
                psv = psb.tile([128, 512], F32, tag="v", name="ps_v", bufs=2)
                for ko in range(8):
                    nc.tensor.matmul(psv[:],
                                     hT[:, ko, rb * 128:(rb + 1) * 128],
                                     wv[:, ko, nh * 512:(nh + 1) * 512],
                                     start=(ko == 0), stop=(ko == 7))
                # heads nh*8 .. nh*8+8 of this row-block
                nc.vector.tensor_copy(
                    vp_rb[:, nh * 8:(nh + 1) * 8, 0:64],
                    psv[:].rearrange("p (h c) -> p h c", c=64))
            nc.vector.memset(vp_rb[:, :, 64:65], 1.0)
            nc.vector.tensor_scalar_mul(Vp[:, rb], Vp[:, rb], valid_t[:, rb:rb + 1])
        psb.release()

        nc.leave_named_scope('B', _scB, False)
        _scC = nc.enter_named_scope('C', False)[0]
        cd = tc.alloc_tile_pool(name="cd", bufs=1, side="left")
        ddw = tc.alloc_tile_pool(name="ddw", bufs=1, side="left")
        O_sb = cd.tile([128, 4, D], BF16, name="O_sb")

        sp = tc.alloc_tile_pool(name="sp", bufs=2, side="right")
        psq_p = tc.alloc_tile_pool(name="psq", bufs=1, space="PSUM")
        psk_p = tc.alloc_tile_pool(name="psk", bufs=2, space="PSUM")
        psc = tc.alloc_tile_pool(name="psc", bufs=2, space="PSUM")
        pso = tc.alloc_tile_pool(name="pso", bufs=1, space="PSUM")

        wpj = None
        for g in range(2):
            wq = wb.tile([128, 8, 512], BF16, tag="wq", name="wq", bufs=2)
            nc.sync.dma_start(
                wq[:], wqkv_d[:, g * 512:(g + 1) * 512]
                .rearrange("(ko p) n -> p ko n", p=128))
            wk = wb.tile([128, 8, 512], BF16, tag="wk", name="wk", bufs=2)
            nc.sync.dma_start(
                wk[:], wqkv_d[:, D + g * 512:D + (g + 1) * 512]
                .rearrange("(ko p) n -> p ko n", p=128))
            for pp in range(4):
                hp = g * 4 + pp
                if hp == 4:
                    # prefetch proj weights while attention still runs
                    wpj = ddw.tile([128, 8, D], BF16, name="wpj")
                    nc.sync.dma_start(wpj[:],
                                      wproj_d.rearrange("(ko p) n -> p ko n", p=128))
                # Q for head-pair hp
                psq = psq_p.tile([128, CHUNK], F32, tag="q", name="ps_q")
                for ko in range(8):
                    nc.tensor.matmul(psq[:], wq[:, ko, pp * 128:(pp + 1) * 128],
                                     hT[:, ko, HALO:ROWS],
                                     start=(ko == 0), stop=(ko == 7))
                if has_qk_bias:
                    nc.scalar.activation(QT[:, hp], psq[:], AF.Identity,
                                         bias=qkb_t[:, 0, hp:hp + 1])
                else:
                    nc.scalar.copy(QT[:, hp], psq[:])
                # K for head-pair hp (two 384-wide chains)
                for n0 in (0, 384):
                    psk = psk_p.tile([128, 384], F32, tag="k", name="ps_k")
                    for ko in range(8):
                        nc.tensor.matmul(psk[:],
                                         wk[:, ko, pp * 128:(pp + 1) * 128],
                                         hT[:, ko, n0:n0 + 384],
                                         start=(ko == 0), stop=(ko == 7))
                    if has_qk_bias:
                        nc.scalar.activation(KT[:, hp, n0:n0 + 384], psk[:],
                                             AF.Identity,
                                             bias=qkb_t[:, 1, hp:hp + 1])
                    else:
                        nc.scalar.copy(KT[:, hp, n0:n0 + 384], psk[:])

                # attention for this head-pair
                S_pr = sp.tile([128, 2, 1536], F32, tag="S", name="S_pr")
                P_pr = sp.tile([128, 2, 1536], PV_DT, tag="P", name="P_pr")
                for kb in range(6):
                    qlo, qhi, il = _kb_span(kb)
                    w = qhi - qlo
                    # [128, 2, 512]: head-halves on PSUM bank boundaries
                    pss = psc.tile([128, 2, 512], F32, tag="s", name="ps_s")
                    for hh in range(2):
                        pb = hh * 64
                        nc.tensor.matmul(pss[:, hh, :w],
                                         KT[pb:pb + 64, hp, kb * 128:(kb + 1) * 128],
                                         QT[pb:pb + 64, hp, qlo:qhi],
                                         start=True, stop=True)
                    nc.vector.tensor_tensor(
                        S_pr[:, :, _KB_OFF[kb]:_KB_OFF[kb] + w],
                        pss[:, :, 0:w],
                        amask_t[:, 2 * hp:2 * hp + 2, il:il + w], ADD)
                # split per head-half so PV of hh=0 starts while hh=1 exps
                nc.scalar.activation(P_pr[:, 0], S_pr[:, 0], AF.Exp)
                nc.scalar.activation(P_pr[:, 1], S_pr[:, 1], AF.Exp)
                for hh in range(2):
                    h_i = hp * 2 + hh
                    # four query-blocks share one PSUM bank: one reciprocal
                    po = pso.tile([128, 4, 65], F32, tag="o", name="ps_o")
                    for qb in range(4):
                        for t in range(3):
                            kb = qb + t
                            qlo, _, _ = _kb_span(kb)
                            pcol = _KB_OFF[kb] + qb * 128 - qlo
                            nc.tensor.matmul(po[:, qb], P_pr[:, hh, pcol:pcol + 128],
                                             Vp[:, kb, h_i * 65:(h_i + 1) * 65],
                                             start=(t == 0), stop=(t == 2))
                    rec = small.tile([128, 4], F32, tag="rec", name="rec")
                    nc.vector.reciprocal(rec[:], po[:, :, 64])
                    for qb in range(4):
                        if qb % 2 == 0:
                            # balance the divide work across ACT and DVE
                            nc.scalar.activation(
                                O_sb[:, qb, h_i * 64:(h_i + 1) * 64],
                                po[:, qb, 0:64], AF.Copy, scale=rec[:, qb:qb + 1])
                        else:
                            nc.vector.tensor_scalar_mul(
                                O_sb[:, qb, h_i * 64:(h_i + 1) * 64],
                                po[:, qb, 0:64], rec[:, qb:qb + 1])
        pso.release()
        psc.release()
        psk_p.release()
        psq_p.release()
        sp.release()
        wb.release()
        bc.release()  # frees hT/QT/KT/Vp/amask

        nc.leave_named_scope('C', _scC, False)
        # ---------------- Phase F pools (right side) ------------------------
        ff = tc.alloc_tile_pool(name="ff", bufs=1, side="right")
        ffT = ff.tile([128, 32, CHUNK], BF16, name="ffT")
        y_sb = ff.tile([128, 4, D], F32, name="y_sb")
        h2T = ff.tile([128, 8, CHUNK], BF16, name="h2T")
        # all of wfc2 stays resident (64 KB/partition in bf16): its DMA runs
        # through D/F1 and fc2 never waits on weights.
        w2all = ff.tile([128, 32, D], BF16, name="w2all")
        w2_r = wfc2_d.rearrange("(fo p) n -> p fo n", p=128)
        if has_fc1_bias:
            b1_t = ff.tile([128, 32], F32, name="b1_t")
            nc.sync.dma_start(b1_t[:], b1_d.rearrange("(fo p) -> p fo", p=128))
        wf = tc.alloc_tile_pool(name="wf", bufs=2, side="right")
        for g in range(2):
            nc.sync.dma_start(w2all[:, g * 2:(g + 1) * 2], w2_r[:, g * 2:(g + 1) * 2])

        # ------- Phase D: O^T + proj + residual + LN2 + h2^T, per qc --------
        _scD = nc.enter_named_scope('D', False)[0]
        dd = tc.alloc_tile_pool(name="dd", bufs=1, side="left")
        OT = dd.tile([128, 8, CHUNK], BF16, name="OT")
        pe_ = tc.alloc_tile_pool(name="pe", bufs=2, side="left")
        psd = tc.alloc_tile_pool(name="psd", bufs=2, space="PSUM")
        h2_pres = []
        for qc in range(4):
            for fc in range(8):
                pst = psd.tile([128, 128], BF16, tag="tr", name="ps_tr2")
                nc.tensor.transpose(pst[:], O_sb[:, qc, fc * 128:(fc + 1) * 128],
                                    ident[:])
                if fc % 2 == 0:
                    nc.vector.tensor_copy(OT[:, fc, qc * 128:(qc + 1) * 128], pst[:])
                else:
                    nc.scalar.copy(OT[:, fc, qc * 128:(qc + 1) * 128], pst[:])
            for nh in range(2):
                psp = psd.tile([128, 512], F32, tag="p", name="ps_p")
                for fc in range(8):
                    nc.tensor.matmul(psp[:], OT[:, fc, qc * 128:(qc + 1) * 128],
                                     wpj[:, fc, nh * 512:(nh + 1) * 512],
                                     start=(fc == 0), stop=(fc == 7))
                nc.vector.tensor_tensor(x2_sb[:, qc, nh * 512:(nh + 1) * 512],
                                        psp[:], xl_sb[:, qc, nh * 512:(nh + 1) * 512],
                                        ADD)
            # LN2 for this chunk runs on DVE/ACT under the next chunk's proj;
            # its PE transposes are deferred below so the in-order PE queue
            # never waits on the serial LN latency.
            h2_pre = pe_.tile([128, D], BF16, tag="h2_pre", name="h2_pre",
                              bufs=4)
            ln_block(tc, x2_sb[:, qc], h2_pre[:], small, dump)
            h2_pres.append(h2_pre)
        for qc in range(4):
            for kc in range(8):
                pst = psd.tile([128, 128], BF16, tag="tr", name="ps_tr2")
                nc.tensor.transpose(pst[:], h2_pres[qc][:, kc * 128:(kc + 1) * 128],
                                    ident[:])
                if kc % 2 == 0:
                    nc.vector.tensor_copy(h2T[:, kc, qc * 128:(qc + 1) * 128], pst[:])
                else:
                    nc.scalar.copy(h2T[:, kc, qc * 128:(qc + 1) * 128], pst[:])
        psd.release()
        pe_.release()
        dd.release()
        ddw.release()
        cd.release()
        xlp.release()

        nc.leave_named_scope('D', _scD, False)
        # ---------------- Phase F1: fc1 + GELU ----------------
        _scF1 = nc.enter_named_scope('F1', False)[0]
        psf = tc.alloc_tile_pool(name="psf", bufs=2, space="PSUM")
        for g in range(8):
            w1 = wf.tile([128, 8, 512], BF16, tag="w1", name="w1", bufs=2)
            nc.sync.dma_start(
                w1[:], wfc1_d[:, g * 512:(g + 1) * 512]
                .rearrange("(ko p) n -> p ko n", p=128))
            # trickle the rest of wfc2 between w1 chunks (4+28 = all 32)
            c = 4 + g * 3
            nw = 3 if g < 7 else 7
            nc.sync.dma_start(w2all[:, c:c + nw], w2_r[:, c:c + nw])
            for f4 in range(4):
                ffc = g * 4 + f4
                psq = psf.tile([128, 512], F32, tag="f", name="ps_f")
                for ko in range(8):
                    nc.tensor.matmul(psq[:], w1[:, ko, f4 * 128:(f4 + 1) * 128],
                                     h2T[:, ko, :], start=(ko == 0), stop=(ko == 7))
                if has_fc1_bias:
                    nc.scalar.activation(ffT[:, ffc, :], psq[:], AF.Gelu,
                                         bias=b1_t[:, ffc:ffc + 1])
                else:
                    nc.scalar.activation(ffT[:, ffc, :], psq[:], AF.Gelu)
        psf.release()

        nc.leave_named_scope('F1', _scF1, False)
        # ---------------- Phase F2: fc2 + residual + store ----------------
        _scF2 = nc.enter_named_scope('F2', False)[0]
        psy = tc.alloc_tile_pool(name="psy", bufs=4, space="PSUM")
        y_dr = y_d.rearrange("(q p) d -> p q d", p=128)
        # qc-outer: each 128-row output chunk finishes its accumulation a
        # quarter of the way in, so its residual add and store overlap the
        # remaining compute instead of piling up at the end.
        for qc in range(4):
            ys = [psy.tile([128, 512], F32, tag="y", name=f"ps_y{nh}")
                  for nh in range(2)]
            for ffc in range(32):
                for nh in range(2):
                    nc.tensor.matmul(ys[nh][:],
                                     ffT[:, ffc, qc * 128:(qc + 1) * 128],
                                     w2all[:, ffc, nh * 512:(nh + 1) * 512],
                                     start=(ffc == 0), stop=(ffc == 31))
            for nh in range(2):
                nc.vector.tensor_tensor(y_sb[:, qc, nh * 512:(nh + 1) * 512],
                                        ys[nh][:],
                                        x2_sb[:, qc, nh * 512:(nh + 1) * 512], ADD)
            nc.sync.dma_start(y_dr[:, qc], y_sb[:, qc])
        psy.release()
        wf.release()
        ff.release()
        de.release()
        small.release()
        glob.release()

        nc.leave_named_scope('F2', _scF2, False)

    nc.compile()
    return nc


def kernel(x, qkv_w, qkv_b, proj_w, proj_b, ln1_g, ln1_b, ln2_g, ln2_b,
           fc1_w, fc1_b, fc2_w, fc2_b):
    from concourse.bass_utils import run_bass_kernel_spmd

    x = np.ascontiguousarray(np.asarray(x, dtype=np.float32))
    f32 = lambda a: np.asarray(a, dtype=np.float32)
    qkv_w, qkv_b = f32(qkv_w), f32(qkv_b)
    proj_w, proj_b = f32(proj_w), f32(proj_b)
    fc1_w, fc1_b = f32(fc1_w), f32(fc1_b)
    fc2_w, fc2_b = f32(fc2_w), f32(fc2_b)
    ln1_g, ln1_b = f32(ln1_g), f32(ln1_b)
    ln2_g, ln2_b = f32(ln2_g), f32(ln2_b)

    # Host-side folding: LN affine into the following weight/bias; HD^-0.5 into Wk.
    import ml_dtypes
    bf = ml_dtypes.bfloat16
    scale = HD ** -0.5
    wqkv = ln1_g[:, None] * qkv_w
    bqkv = qkv_b + ln1_b @ qkv_w
    wqkv = np.ascontiguousarray(wqkv)
    wqkv[:, D:2 * D] *= scale
    bqkv = bqkv.copy()
    bqkv[D:2 * D] *= scale
    wfc1 = np.ascontiguousarray(ln2_g[:, None] * fc1_w)
    bfc1 = fc1_b + ln2_b @ fc1_w
    wqkv = np.ascontiguousarray(wqkv.astype(bf))
    wproj16 = np.ascontiguousarray(proj_w.astype(bf))
    wfc1 = np.ascontiguousarray(wfc1.astype(bf))
    wfc216 = np.ascontiguousarray(fc2_w.astype(bf))

    if np.any(bqkv[2 * D:]) or np.any(proj_b) or np.any(fc2_b):
        raise NotImplementedError("nonzero v/proj/fc2 bias not supported")

    has_qk_bias = bool(np.any(bqkv[:2 * D]))
    has_fc1_bias = bool(np.any(bfc1))
    key = (has_qk_bias, has_fc1_bias)
    if key not in _cache:
        _cache[key] = _build_program(*key)
    nc = _cache[key]

    amask = _build_amask()
    in_maps = []
    for c in range(NCORES):
        b, ck = c // 4, c % 4
        g0 = ck * CHUNK
        xl = np.ascontiguousarray(x[b, g0:g0 + CHUNK].astype(bf))
        if ck > 0:
            xhalo = np.ascontiguousarray(x[b, g0 - HALO:g0].astype(bf))
        else:
            xhalo = np.zeros((HALO, D), bf)
        valid = np.ones((6, 128), np.float32)
        if ck == 0:
            valid[:2] = 0.0
        m = {"xh": xhalo, "xl": xl, "wqkv": wqkv, "wproj": wproj16,
             "wfc1": wfc1, "wfc2": wfc216, "amask": amask, "valid": valid}
        if has_qk_bias:
            m["qkbias"] = np.ascontiguousarray(
                bqkv[:2 * D].reshape(2, 8, 128))
        if has_fc1_bias:
            m["b1"] = bfc1
        in_maps.append(m)

    res = run_bass_kernel_spmd(nc, in_maps, core_ids=list(range(NCORES)))
    y = np.empty((B, N, D), np.float32)
    for c in range(NCORES):
        b, ck = c // 4, c % 4
        y[b, ck * CHUNK:(ck + 1) * CHUNK] = res.results[c]["y"]
    return y


# revision 55
# speedup vs baseline: 1.1204x; 1.0019x over previous
"""Trainium2 Bass kernel for nn_AdvancedTransformerBlock_15006615733156.

Pre-norm transformer block: LN1 -> QKV -> sliding-window causal attention with
ALiBi (window 256) -> proj residual -> LN2 -> FFN (exact GELU) residual.
B=2, N=2048, D=1024, H=16, HD=64.

Sharding: 8 cores = batch(2) x sequence(4 chunks of 512 rows). The 256-wide
sliding window means each core only needs a 256-row halo of x before its
chunk — no collectives. Chunk-0 cores get a zeroed halo plus a `valid` mask
that zeroes halo V' rows (kills both numerator and softmax denominator).

On-chip layout: scores are computed transposed (S_t[kj, qi]) so the
probability tile is directly consumable as matmul lhsT for PV; the softmax
denominator comes from an appended ones-column in V'. All big matmuls run in
bf16 (fast-weight-load on LDWEIGHTS, low PE power); accumulation stays fp32
in PSUM, and LN stats / residual adds stay fp32.

Phase structure is chosen to keep the Tensor engine densely fed: the HW
activity manager halves the PE duty limit (k=4/8) within ~10us of the PE
going idle, so PE-sparse phases run their matmuls at half rate. QKV
projections are therefore interleaved per-head-pair with that head-pair's
attention, and LN2 is folded into the proj/residual loop.
"""
import sys, math, os
sys.path.insert(0, '/opt/trn_rl_repo')
import numpy as np

B, N, D, H, HD, WIN = 2, 2048, 1024, 16, 64, 256
CHUNK, HALO, ROWS = 512, 256, 768
NEG = -1e30
LN_EPS = 1e-5
NCORES = 8

_cache = {}


def _alibi_slopes(n):
    closest = 2 ** math.floor(math.log2(n))
    base = 2.0 ** (-(2.0 ** (-(math.log2(closest) - 3))))
    return np.power(base, np.arange(1, closest + 1)).astype(np.float32)


def _build_amask():
    """Additive pre-softmax bias, [128, H, 384] bf16 (partition-major so the
    DMA moves one 12 KB contiguous run per partition).

    Softmax over keys j is invariant to any per-query-column constant, so the
    reference's "+1 inside window" and the -slope*i part of the ALiBi term
    drop out; what remains is slope*(j - i) <= 0 inside the band, -1e30
    outside. Values near each column's max are near zero, so bf16's relative
    rounding cannot disturb the softmax weights meaningfully.
    """
    import ml_dtypes
    sl = _alibi_slopes(H)
    j = np.arange(128)[:, None]
    i = np.arange(384)[None, :]
    band = ((i - j) >= 0) & ((i - j) <= 255)
    out = np.where(band[None], sl[:, None, None] * (j - i)[None], NEG)
    return np.ascontiguousarray(
        out.astype(ml_dtypes.bfloat16).transpose(1, 0, 2))


def _kb_span(kb):
    qlo = max(0, kb * 128 - 256)
    qhi = min(512, kb * 128 + 128)
    return qlo, qhi, qlo - (kb * 128 - 256)


_KB_OFF = [0]
for _kb in range(6):
    _q0, _q1, _ = _kb_span(_kb)
    _KB_OFF.append(_KB_OFF[-1] + (_q1 - _q0))  # offsets into the 1536-wide S row


def _build_program(has_qk_bias, has_fc1_bias):
    import concourse.bass as bass
    import concourse.tile as tile
    from concourse import bacc, mybir
    from concourse.masks import make_identity

    F32 = mybir.dt.float32
    BF16 = mybir.dt.bfloat16
    # 16-bit P/V' runs PV at 1 cyc/row and gets FWL on the weight load; fp16's
    # 10-bit mantissa keeps softmax-prob rounding at ~5e-4.
    _pv = os.environ.get("K_PV_DT", "f16")
    PV_DT = {"f32": F32, "bf16": BF16, "f16": mybir.dt.float16}[_pv]
    AF = mybir.ActivationFunctionType
    ADD, MULT = mybir.AluOpType.add, mybir.AluOpType.mult

    nc = bacc.Bacc("TRN2", target_bir_lowering=False, debug=False,
                   num_devices=NCORES)

    xh_d = nc.dram_tensor("xh", [HALO, D], BF16, kind="ExternalInput").ap()
    xl_d = nc.dram_tensor("xl", [CHUNK, D], BF16, kind="ExternalInput").ap()
    wqkv_d = nc.dram_tensor("wqkv", [D, 3 * D], BF16, kind="ExternalInput").ap()
    wproj_d = nc.dram_tensor("wproj", [D, D], BF16, kind="ExternalInput").ap()
    wfc1_d = nc.dram_tensor("wfc1", [D, 4 * D], BF16, kind="ExternalInput").ap()
    wfc2_d = nc.dram_tensor("wfc2", [4 * D, D], BF16, kind="ExternalInput").ap()
    amask_d = nc.dram_tensor("amask", [128, H, 384], BF16, kind="ExternalInput").ap()
    valid_d = nc.dram_tensor("valid", [6, 128], F32, kind="ExternalInput").ap()
    if has_qk_bias:
        qkb_d = nc.dram_tensor("qkbias", [2, 8, 128], F32, kind="ExternalInput").ap()
    if has_fc1_bias:
        b1_d = nc.dram_tensor("b1", [4 * D], F32, kind="ExternalInput").ap()
    y_d = nc.dram_tensor("y", [CHUNK, D], F32, kind="ExternalOutput").ap()

    def ln_block(tc, x_ap, out_ap, small, dump):
        """LayerNorm (no affine) of [128, 1024]: out = (x - mu) * rstd.

        Stats split across engines in parallel: ACT computes E[x^2] via
        Square+accum while DVE reduces E[x]; var = E[x^2] - mu^2.
        """
        sq = small.tile([128, 1], F32, tag="sq", name="sq")
        nc.scalar.activation(dump[:], x_ap, AF.Square, accum_out=sq[:])
        sums = small.tile([128, 1], F32, tag="sums", name="sums")
        nc.vector.tensor_reduce(sums[:], x_ap, mybir.AxisListType.X,
                                mybir.AluOpType.add)
        negmu = small.tile([128, 1], F32, tag="negmu", name="negmu")
        nc.vector.tensor_scalar_mul(negmu[:], sums[:], -1.0 / D)
        m2 = small.tile([128, 1], F32, tag="m2", name="m2")
        nc.vector.tensor_tensor(m2[:], negmu[:], negmu[:], MULT)
        bvar = small.tile([128, 1], F32, tag="bvar", name="bvar")
        nc.vector.tensor_scalar(bvar[:], m2[:], -1.0, LN_EPS, MULT, ADD)
        st = small.tile([128, 1], F32, tag="st", name="st")
        nc.scalar.activation(st[:], sq[:], AF.Sqrt, bias=bvar[:], scale=1.0 / D)
        rstd = small.tile([128, 1], F32, tag="rstd", name="rstd")
        nc.vector.reciprocal(rstd[:], st[:])
        nmr = small.tile([128, 1], F32, tag="nmr", name="nmr")
        nc.vector.tensor_tensor(nmr[:], negmu[:], rstd[:], MULT)
        nc.vector.tensor_scalar(out_ap, x_ap, rstd[:], nmr[:], MULT, ADD)

    with tile.TileContext(nc) as tc:
        # Pool lifetimes form two LIFO stacks (SBUF left/right).
        glob = tc.alloc_tile_pool(name="glob", bufs=1, side="right")
        small = tc.alloc_tile_pool(name="small", bufs=8, side="right")
        de = tc.alloc_tile_pool(name="de", bufs=1, side="right")  # x2 (D..end)

        ident = glob.tile([128, 128], BF16, name="ident")
        make_identity(nc, ident[:])
        dump = glob.tile([128, D], BF16, name="dump")
        x2_sb = de.tile([128, 4, D], F32, name="x2_sb")

        # bc: everything attention-scoped (released at end of attention)
        bc = tc.alloc_tile_pool(name="bc", bufs=1, side="right")
        hT = bc.tile([128, 8, ROWS], BF16, name="hT")
        QT = bc.tile([128, 8, CHUNK], BF16, name="QT")     # [hd-pair, pair, qi]
        KT = bc.tile([128, 8, ROWS], BF16, name="KT")
        Vp = bc.tile([128, 6, 16 * 65], PV_DT, name="Vp")  # per-head 65-col groups
        amask_t = bc.tile([128, H, 384], BF16, name="amask_t")
        valid_t = bc.tile([128, 6], F32, name="valid_t")

        # ---------------- Phase A: LN1 + h^T ----------------
        _scA = nc.enter_named_scope('A', False)[0]
        xlp = tc.alloc_tile_pool(name="xlp", bufs=1, side="left")  # x local, A..D
        pa = tc.alloc_tile_pool(name="pa", bufs=2, side="left")
        psa = tc.alloc_tile_pool(name="psa", bufs=2, space="PSUM")
        xh_sb = pa.tile([128, 2, D], BF16, name="xh_sb", bufs=1)
        # halo first (LN block 0 needs it; HWDGE DMAs are FIFO), split so
        # block 0 lands early
        xh_r = xh_d.rearrange("(q p) d -> p q d", p=128)
        nc.sync.dma_start(xh_sb[:, 0], xh_r[:, 0])
        nc.sync.dma_start(xh_sb[:, 1], xh_r[:, 1])
        xl_sb = xlp.tile([128, 4, D], BF16, name="xl_sb")
        xl_r = xl_d.rearrange("(q p) d -> p q d", p=128)
        for q in range(4):
            nc.sync.dma_start(xl_sb[:, q], xl_r[:, q])
        nc.sync.dma_start(valid_t[:], valid_d.rearrange("k p -> p k"))
        # amask is only needed by the softmax; keep it off the SP DGE queue so
        # the first weight chunks aren't stuck behind its 1.5 MB.
        nc.scalar.dma_start(amask_t[:], amask_d)
        if has_qk_bias:
            qkb_t = bc.tile([128, 2, 8], F32, name="qkb_t")
            nc.scalar.dma_start(qkb_t[:], qkb_d.rearrange("t g p -> p t g"))
        for blk in range(6):
            x_ap = xh_sb[:, blk] if blk < 2 else xl_sb[:, blk - 2]
            h_pre = pa.tile([128, D], BF16, tag="h_pre", name="h_pre")
            ln_block(tc, x_ap, h_pre[:], small, dump)
            for kc in range(8):
                pst = psa.tile([128, 128], BF16, tag="tr", name="ps_tr")
                nc.tensor.transpose(pst[:], h_pre[:, kc * 128:(kc + 1) * 128],
                                    ident[:])
                if kc % 2 == 0:
                    nc.vector.tensor_copy(hT[:, kc, blk * 128:(blk + 1) * 128], pst[:])
                else:
                    nc.scalar.copy(hT[:, kc, blk * 128:(blk + 1) * 128], pst[:])
        psa.release()
        pa.release()

        nc.leave_named_scope('A', _scA, False)
        # ---------------- Phase B+C: V, then per-head-pair QK + attention ----
        # Interleaving QK projections with attention keeps the PE dense, so
        # the activity manager never drops it to half duty mid-attention.
        _scB = nc.enter_named_scope('B', False)[0]
        wb = tc.alloc_tile_pool(name="wb", bufs=2, side="right")
        psb = tc.alloc_tile_pool(name="psb", bufs=2, space="PSUM")

        # V first: its compute covers the Q/K weight DMAs
        wv = wb.tile([128, 8, D], BF16, tag="wv", name="wv", bufs=1)
        nc.sync.dma_start(wv[:],
                          wqkv_d[:, 2 * D:3 * D].rearrange("(ko p) n -> p ko n", p=128))
        for rb in range(6):
            vp_rb = Vp[:, rb].rearrange("p (h c) -> p h c", c=65)
            for nh in range(2):
                psv = psb.tile([128, 512], F32, tag="v", name="ps_v", bufs=2)
                for ko in range(8):
                    nc.tensor.matmul(psv[:],
                                     hT[:, ko, rb * 128:(rb + 1) * 128],
                                     wv[:, ko, nh * 512:(nh + 1) * 512],
                                     start=(ko == 0), stop=(ko == 7))
                # heads nh*8 .. nh*8+8 of this row-block
                nc.vector.tensor_copy(
                    vp_rb[:, nh * 8:(nh + 1) * 8, 0:64],
                    psv[:].rearrange("p (h c) -> p h c", c=64))
            nc.vector.memset(vp_rb[:, :, 64:65], 1.0)
            nc.vector.tensor_scalar_mul(Vp[:, rb], Vp[:, rb], valid_t[:, rb:rb + 1])
        psb.release()

        nc.leave_named_scope('B', _scB, False)
        _scC = nc.enter_named_scope('C', False)[0]
        cd = tc.alloc_tile_pool(name="cd", bufs=1, side="left")
        ddw = tc.alloc_tile_pool(name="ddw", bufs=1, side="left")
        O_sb = cd.tile([128, 4, D], BF16, name="O_sb")

        sp = tc.alloc_tile_pool(name="sp", bufs=2, side="right")
        psq_p = tc.alloc_tile_pool(name="psq", bufs=1, space="PSUM")
        psk_p = tc.alloc_tile_pool(name="psk", bufs=2, space="PSUM")
        psc = tc.alloc_tile_pool(name="psc", bufs=2, space="PSUM")
        pso = tc.alloc_tile_pool(name="pso", bufs=1, space="PSUM")

        wpj = None
        for g in range(2):
            wq = wb.tile([128, 8, 512], BF16, tag="wq", name="wq", bufs=2)
            nc.sync.dma_start(
                wq[:], wqkv_d[:, g * 512:(g + 1) * 512]
                .rearrange("(ko p) n -> p ko n", p=128))
            wk = wb.tile([128, 8, 512], BF16, tag="wk", name="wk", bufs=2)
            nc.sync.dma_start(
                wk[:], wqkv_d[:, D + g * 512:D + (g + 1) * 512]
                .rearrange("(ko p) n -> p ko n", p=128))
            for pp in range(4):
                hp = g * 4 + pp
                if hp == 4:
                    # prefetch proj weights while attention still runs
                    wpj = ddw.tile([128, 8, D], BF16, name="wpj")
                    nc.sync.dma_start(wpj[:],
                                      wproj_d.rearrange("(ko p) n -> p ko n", p=128))
                # Q for head-pair hp
                psq = psq_p.tile([128, CHUNK], F32, tag="q", name="ps_q")
                for ko in range(8):
                    nc.tensor.matmul(psq[:], wq[:, ko, pp * 128:(pp + 1) * 128],
                                     hT[:, ko, HALO:ROWS],
                                     start=(ko == 0), stop=(ko == 7))
                if has_qk_bias:
                    nc.scalar.activation(QT[:, hp], psq[:], AF.Identity,
                                         bias=qkb_t[:, 0, hp:hp + 1])
                else:
                    nc.scalar.copy(QT[:, hp], psq[:])
                # K for head-pair hp (two 384-wide chains)
                for n0 in (0, 384):
                    psk = psk_p.tile([128, 384], F32, tag="k", name="ps_k")
                    for ko in range(8):
                        nc.tensor.matmul(psk[:],
                                         wk[:, ko, pp * 128:(pp + 1) * 128],
                                         hT[:, ko, n0:n0 + 384],
                                         start=(ko == 0), stop=(ko == 7))
                    if has_qk_bias:
                        nc.scalar.activation(KT[:, hp, n0:n0 + 384], psk[:],
                                             AF.Identity,
                                             bias=qkb_t[:, 1, hp:hp + 1])
                    else:
                        nc.scalar.copy(KT[:, hp, n0:n0 + 384], psk[:])

                # attention for this head-pair
                S_pr = sp.tile([128, 2, 1536], F32, tag="S", name="S_pr")
                P_pr = sp.tile([128, 2, 1536], PV_DT, tag="P", name="P_pr")
                for kb in range(6):
                    qlo, qhi, il = _kb_span(kb)
                    w = qhi - qlo
                    # [128, 2, 512]: head-halves on PSUM bank boundaries
                    pss = psc.tile([128, 2, 512], F32, tag="s", name="ps_s")
                    for hh in range(2):
                        pb = hh * 64
                        nc.tensor.matmul(pss[:, hh, :w],
                                         KT[pb:pb + 64, hp, kb * 128:(kb + 1) * 128],
                                         QT[pb:pb + 64, hp, qlo:qhi],
                                         start=True, stop=True)
                    nc.vector.tensor_tensor(
                        S_pr[:, :, _KB_OFF[kb]:_KB_OFF[kb] + w],
                        pss[:, :, 0:w],
                        amask_t[:, 2 * hp:2 * hp + 2, il:il + w], ADD)
                # split per head-half so PV of hh=0 starts while hh=1 exps
                nc.scalar.activation(P_pr[:, 0], S_pr[:, 0], AF.Exp)
                nc.scalar.activation(P_pr[:, 1], S_pr[:, 1], AF.Exp)
                for hh in range(2):
                    h_i = hp * 2 + hh
                    # four query-blocks share one PSUM bank: one reciprocal
                    po = pso.tile([128, 4, 65], F32, tag="o", name="ps_o")
                    for qb in range(4):
                        for t in range(3):
                            kb = qb + t
                            qlo, _, _ = _kb_span(kb)
                            pcol = _KB_OFF[kb] + qb * 128 - qlo
                            nc.tensor.matmul(po[:, qb], P_pr[:, hh, pcol:pcol + 128],
                                             Vp[:, kb, h_i * 65:(h_i + 1) * 65],
                                             start=(t == 0), stop=(t == 2))
                    rec = small.tile([128, 4], F32, tag="rec", name="rec")
                    nc.vector.reciprocal(rec[:], po[:, :, 64])
                    for qb in range(4):
                        if qb % 2 == 0:
                            # balance the divide work across ACT and DVE
                            nc.scalar.activation(
                                O_sb[:, qb, h_i * 64:(h_i + 1) * 64],
                                po[:, qb, 0:64], AF.Copy, scale=rec[:, qb:qb + 1])
                        else:
                            nc.vector.tensor_scalar_mul(
                                O_sb[:, qb, h_i * 64:(h_i + 1) * 64],
                                po[:, qb, 0:64], rec[:, qb:qb + 1])
        pso.release()
        psc.release()
        psk_p.release()
        psq_p.release()
        sp.release()
        wb.release()
        bc.release()  # frees hT/QT/KT/Vp/amask

        nc.leave_named_scope('C', _scC, False)
        # ---------------- Phase F pools (right side) ------------------------
        ff = tc.alloc_tile_pool(name="ff", bufs=1, side="right")
        ffT = ff.tile([128, 32, CHUNK], BF16, name="ffT")
        y_sb = ff.tile([128, 4, D], F32, name="y_sb")
        h2T = ff.tile([128, 8, CHUNK], BF16, name="h2T")
        # all of wfc2 stays resident (64 KB/partition in bf16): its DMA runs
        # through D/F1 and fc2 never waits on weights.
        w2all = ff.tile([128, 32, D], BF16, name="w2all")
        w2_r = wfc2_d.rearrange("(fo p) n -> p fo n", p=128)
        if has_fc1_bias:
            b1_t = ff.tile([128, 32], F32, name="b1_t")
            nc.sync.dma_start(b1_t[:], b1_d.rearrange("(fo p) -> p fo", p=128))
        wf = tc.alloc_tile_pool(name="wf", bufs=2, side="right")
        for g in range(2):
            nc.sync.dma_start(w2all[:, g * 2:(g + 1) * 2], w2_r[:, g * 2:(g + 1) * 2])

        # ------- Phase D: O^T + proj + residual + LN2 + h2^T, per qc --------
        _scD = nc.enter_named_scope('D', False)[0]
        dd = tc.alloc_tile_pool(name="dd", bufs=1, side="left")
        OT = dd.tile([128, 8, CHUNK], BF16, name="OT")
        pe_ = tc.alloc_tile_pool(name="pe", bufs=2, side="left")
        psd = tc.alloc_tile_pool(name="psd", bufs=2, space="PSUM")
        h2_pres = []
        for qc in range(4):
            for fc in range(8):
                pst = psd.tile([128, 128], BF16, tag="tr", name="ps_tr2")
                nc.tensor.transpose(pst[:], O_sb[:, qc, fc * 128:(fc + 1) * 128],
                                    ident[:])
                if fc % 2 == 0:
                    nc.vector.tensor_copy(OT[:, fc, qc * 128:(qc + 1) * 128], pst[:])
                else:
                    nc.scalar.copy(OT[:, fc, qc * 128:(qc + 1) * 128], pst[:])
            for nh in range(2):
                psp = psd.tile([128, 512], F32, tag="p", name="ps_p")
                for fc in range(8):
                    nc.tensor.matmul(psp[:], OT[:, fc, qc * 128:(qc + 1) * 128],
                                     wpj[:, fc, nh * 512:(nh + 1) * 512],
                                     start=(fc == 0), stop=(fc == 7))
                nc.vector.tensor_tensor(x2_sb[:, qc, nh * 512:(nh + 1) * 512],
                                        psp[:], xl_sb[:, qc, nh * 512:(nh + 1) * 512],
                                        ADD)
            # LN2 for this chunk runs on DVE/ACT under the next chunk's proj;
            # its PE transposes are deferred below so the in-order PE queue
            # never waits on the serial LN latency.
            h2_pre = pe_.tile([128, D], BF16, tag="h2_pre", name="h2_pre",
                              bufs=4)
            ln_block(tc, x2_sb[:, qc], h2_pre[:], small, dump)
            h2_pres.append(h2_pre)
        for qc in range(4):
            for kc in range(8):
                pst = psd.tile([128, 128], BF16, tag="tr", name="ps_tr2")
                nc.tensor.transpose(pst[:], h2_pres[qc][:, kc * 128:(kc + 1) * 128],
                                    ident[:])
                if kc % 2 == 0:
                    nc.vector.tensor_copy(h2T[:, kc, qc * 128:(qc + 1) * 128], pst[:])
                else:
                    nc.scalar.copy(h2T[:, kc, qc * 128:(qc + 1) * 128], pst[:])
        psd.release()
        pe_.release()
        dd.release()
        ddw.release()
        cd.release()
        xlp.release()

        nc.leave_named_scope('D', _scD, False)
        # ---------------- Phase F1: fc1 + GELU ----------------
        _scF1 = nc.enter_named_scope('F1', False)[0]
        psf = tc.alloc_tile_pool(name="psf", bufs=2, space="PSUM")
        for g in range(8):
            w1 = wf.tile([128, 8, 512], BF16, tag="w1", name="w1", bufs=2)
            nc.sync.dma_start(
                w1[:], wfc1_d[:, g * 512:(g + 1) * 512]
                .rearrange("(ko p) n -> p ko n", p=128))
            # trickle the rest of wfc2 between w1 chunks (4+28 = all 32)
            c = 4 + g * 3
            nw = 3 if g < 7 else 7
            nc.sync.dma_start(w2all[:, c:c + nw], w2_r[:, c:c + nw])
            for f4 in range(4):
                ffc = g * 4 + f4
                psq = psf.tile([128, 512], F32, tag="f", name="ps_f")
                for ko in range(8):
                    nc.tensor.matmul(psq[:], w1[:, ko, f4 * 128:(f4 + 1) * 128],
                                     h2T[:, ko, :], start=(ko == 0), stop=(ko == 7))
                if has_fc1_bias:
                    nc.scalar.activation(ffT[:, ffc, :], psq[:], AF.Gelu,
                                         bias=b1_t[:, ffc:ffc + 1])
                else:
                    nc.scalar.activation(ffT[:, ffc, :], psq[:], AF.Gelu)
        psf.release()

        nc.leave_named_scope('F1', _scF1, False)
        # ---------------- Phase F2: fc2 + residual + store ----------------
        _scF2 = nc.enter_named_scope('F2', False)[0]
        psy = tc.alloc_tile_pool(name="psy", bufs=4, space="PSUM")
        y_dr = y_d.rearrange("(q p) d -> p q d", p=128)
        # qc-outer: each 128-row output chunk finishes its accumulation a
        # quarter of the way in, so its residual add and store overlap the
        # remaining compute instead of piling up at the end.
        for qc in range(4):
            ys = [psy.tile([128, 512], F32, tag="y", name=f"ps_y{nh}")
                  for nh in range(2)]
            for ffc in range(32):
                for nh in range(2):
                    nc.tensor.matmul(ys[nh][:],
                                     ffT[:, ffc, qc * 128:(qc + 1) * 128],
                                     w2all[:, ffc, nh * 512:(nh + 1) * 512],
                                     start=(ffc == 0), stop=(ffc == 31))
            for nh in range(2):
                nc.vector.tensor_tensor(y_sb[:, qc, nh * 512:(nh + 1) * 512],
                                        ys[nh][:],
                                        x2_sb[:, qc, nh * 512:(nh + 1) * 512], ADD)
            nc.sync.dma_start(y_dr[:, qc], y_sb[:, qc])
        psy.release()
        wf.release()
        ff.release()
        de.release()
        small.release()
        glob.release()

        nc.leave_named_scope('F2', _scF2, False)

    nc.compile()
    return nc


def kernel(x, qkv_w, qkv_b, proj_w, proj_b, ln1_g, ln1_b, ln2_g, ln2_b,
           fc1_w, fc1_b, fc2_w, fc2_b):
    from concourse.bass_utils import run_bass_kernel_spmd

    x = np.ascontiguousarray(np.asarray(x, dtype=np.float32))
    f32 = lambda a: np.asarray(a, dtype=np.float32)
    qkv_w, qkv_b = f32(qkv_w), f32(qkv_b)
    proj_w, proj_b = f32(proj_w), f32(proj_b)
    fc1_w, fc1_b = f32(fc1_w), f32(fc1_b)
    fc2_w, fc2_b = f32(fc2_w), f32(fc2_b)
    ln1_g, ln1_b = f32(ln1_g), f32(ln1_b)
    ln2_g, ln2_b = f32(ln2_g), f32(ln2_b)

    # Host-side folding: LN affine into the following weight/bias; HD^-0.5 into Wk.
    import ml_dtypes
    bf = ml_dtypes.bfloat16
    scale = HD ** -0.5
    wqkv = ln1_g[:, None] * qkv_w
    bqkv = qkv_b + ln1_b @ qkv_w
    wqkv = np.ascontiguousarray(wqkv)
    wqkv[:, D:2 * D] *= scale
    bqkv = bqkv.copy()
    bqkv[D:2 * D] *= scale
    wfc1 = np.ascontiguousarray(ln2_g[:, None] * fc1_w)
    bfc1 = fc1_b + ln2_b @ fc1_w
    wqkv = np.ascontiguousarray(wqkv.astype(bf))
    wproj16 = np.ascontiguousarray(proj_w.astype(bf))
    wfc1 = np.ascontiguousarray(wfc1.astype(bf))
    wfc216 = np.ascontiguousarray(fc2_w.astype(bf))

    if np.any(bqkv[2 * D:]) or np.any(proj_b) or np.any(fc2_b):
        raise NotImplementedError("nonzero v/proj/fc2 bias not supported")

    has_qk_bias = bool(np.any(bqkv[:2 * D]))
    has_fc1_bias = bool(np.any(bfc1))
    key = (has_qk_bias, has_fc1_bias)
    if key not in _cache:
        _cache[key] = _build_program(*key)
    nc = _cache[key]

    amask = _build_amask()
    in_maps = []
    for c in range(NCORES):
        b, ck = c // 4, c % 4
        g0 = ck * CHUNK
        xl = np.ascontiguousarray(x[b, g0:g0 + CHUNK].astype(bf))
        if ck > 0:
            xhalo = np.ascontiguousarray(x[b, g0 - HALO:g0].astype(bf))
        else:
            xhalo = np.zeros((HALO, D), bf)
        valid = np.ones((6, 128), np.float32)
        if ck == 0:
            valid[:2] = 0.0
        m = {"xh": xhalo, "xl": xl, "wqkv": wqkv, "wproj": wproj16,
             "wfc1": wfc1, "wfc2": wfc216, "amask": amask, "valid": valid}
        if has_qk_bias:
            m["qkbias"] = np.ascontiguousarray(
                bqkv[:2 * D].reshape(2, 8, 128))
        if has_fc1_bias:
            m["b1"] = bfc1
        in_maps.append(m)

    res = run_bass_kernel_spmd(nc, in_maps, core_ids=list(range(NCORES)))
    y = np.empty((B, N, D), np.float32)
    for c in range(NCORES):
        b, ck = c // 4, c % 4
        y[b, ck * CHUNK:(ck + 1) * CHUNK] = res.results[c]["y"]
    return y


# revision 57
# speedup vs baseline: 1.1561x; 1.0319x over previous
"""Trainium2 Bass kernel for nn_AdvancedTransformerBlock_15006615733156.

Pre-norm transformer block: LN1 -> QKV -> sliding-window causal attention with
ALiBi (window 256) -> proj residual -> LN2 -> FFN (exact GELU) residual.
B=2, N=2048, D=1024, H=16, HD=64.

Sharding: 8 cores = batch(2) x sequence(4 chunks of 512 rows). The 256-wide
sliding window means each core only needs a 256-row halo of x before its
chunk — no collectives. Chunk-0 cores get a zeroed halo plus a `valid` mask
that zeroes halo V' rows (kills both numerator and softmax denominator).

On-chip layout: scores are computed transposed (S_t[kj, qi]) so the
probability tile is directly consumable as matmul lhsT for PV; the softmax
denominator comes from an appended ones-column in V'. All big matmuls run in
bf16 (fast-weight-load on LDWEIGHTS, low PE power); accumulation stays fp32
in PSUM, and LN stats / residual adds stay fp32.

Phase structure is chosen to keep the Tensor engine densely fed: the HW
activity manager halves the PE duty limit (k=4/8) within ~10us of the PE
going idle, so PE-sparse phases run their matmuls at half rate. QKV
projections are therefore interleaved per-head-pair with that head-pair's
attention, and LN2 is folded into the proj/residual loop.
"""
import sys, math, os
sys.path.insert(0, '/opt/trn_rl_repo')
import numpy as np

B, N, D, H, HD, WIN = 2, 2048, 1024, 16, 64, 256
CHUNK, HALO, ROWS = 512, 256, 768
NEG = -1e30
LN_EPS = 1e-5
NCORES = 8

_cache = {}


def _alibi_slopes(n):
    closest = 2 ** math.floor(math.log2(n))
    base = 2.0 ** (-(2.0 ** (-(math.log2(closest) - 3))))
    return np.power(base, np.arange(1, closest + 1)).astype(np.float32)


def _build_amask():
    """Additive pre-softmax bias, [128, H, 384] bf16 (partition-major so the
    DMA moves one 12 KB contiguous run per partition).

    Softmax over keys j is invariant to any per-query-column constant, so the
    reference's "+1 inside window" and the -slope*i part of the ALiBi term
    drop out; what remains is slope*(j - i) <= 0 inside the band, -1e30
    outside. Values near each column's max are near zero, so bf16's relative
    rounding cannot disturb the softmax weights meaningfully.
    """
    import ml_dtypes
    sl = _alibi_slopes(H)
    j = np.arange(128)[:, None]
    i = np.arange(384)[None, :]
    band = ((i - j) >= 0) & ((i - j) <= 255)
    out = np.where(band[None], sl[:, None, None] * (j - i)[None], NEG)
    return np.ascontiguousarray(
        out.astype(ml_dtypes.bfloat16).transpose(1, 0, 2))


def _kb_span(kb):
    qlo = max(0, kb * 128 - 256)
    qhi = min(512, kb * 128 + 128)
    return qlo, qhi, qlo - (kb * 128 - 256)


_KB_OFF = [0]
for _kb in range(6):
    _q0, _q1, _ = _kb_span(_kb)
    _KB_OFF.append(_KB_OFF[-1] + (_q1 - _q0))  # offsets into the 1536-wide S row


def _build_program(has_qk_bias, has_fc1_bias):
    import concourse.bass as bass
    import concourse.tile as tile
    from concourse import bacc, mybir
    from concourse.masks import make_identity

    F32 = mybir.dt.float32
    BF16 = mybir.dt.bfloat16
    # 16-bit P/V' runs PV at 1 cyc/row and gets FWL on the weight load; fp16's
    # 10-bit mantissa keeps softmax-prob rounding at ~5e-4.
    _pv = os.environ.get("K_PV_DT", "f16")
    PV_DT = {"f32": F32, "bf16": BF16, "f16": mybir.dt.float16}[_pv]
    AF = mybir.ActivationFunctionType
    ADD, MULT = mybir.AluOpType.add, mybir.AluOpType.mult

    nc = bacc.Bacc("TRN2", target_bir_lowering=False, debug=False,
                   num_devices=NCORES)

    xh_d = nc.dram_tensor("xh", [HALO, D], BF16, kind="ExternalInput").ap()
    xl_d = nc.dram_tensor("xl", [CHUNK, D], BF16, kind="ExternalInput").ap()
    wqkv_d = nc.dram_tensor("wqkv", [D, 3 * D], BF16, kind="ExternalInput").ap()
    wproj_d = nc.dram_tensor("wproj", [D, D], BF16, kind="ExternalInput").ap()
    wfc1_d = nc.dram_tensor("wfc1", [D, 4 * D], BF16, kind="ExternalInput").ap()
    wfc2_d = nc.dram_tensor("wfc2", [4 * D, D], BF16, kind="ExternalInput").ap()
    amask_d = nc.dram_tensor("amask", [128, H, 384], BF16, kind="ExternalInput").ap()
    valid_d = nc.dram_tensor("valid", [6, 128], F32, kind="ExternalInput").ap()
    if has_qk_bias:
        qkb_d = nc.dram_tensor("qkbias", [2, 8, 128], F32, kind="ExternalInput").ap()
    if has_fc1_bias:
        b1_d = nc.dram_tensor("b1", [4 * D], F32, kind="ExternalInput").ap()
    y_d = nc.dram_tensor("y", [CHUNK, D], F32, kind="ExternalOutput").ap()

    def ln_block(tc, x_ap, out_ap, small, dump):
        """LayerNorm (no affine) of [128, 1024]: out = (x - mu) * rstd.

        Stats split across engines in parallel: ACT computes E[x^2] via
        Square+accum while DVE reduces E[x]; var = E[x^2] - mu^2.
        """
        sq = small.tile([128, 1], F32, tag="sq", name="sq")
        nc.scalar.activation(dump[:], x_ap, AF.Square, accum_out=sq[:])
        sums = small.tile([128, 1], F32, tag="sums", name="sums")
        nc.vector.tensor_reduce(sums[:], x_ap, mybir.AxisListType.X,
                                mybir.AluOpType.add)
        negmu = small.tile([128, 1], F32, tag="negmu", name="negmu")
        nc.vector.tensor_scalar_mul(negmu[:], sums[:], -1.0 / D)
        m2 = small.tile([128, 1], F32, tag="m2", name="m2")
        nc.vector.tensor_tensor(m2[:], negmu[:], negmu[:], MULT)
        bvar = small.tile([128, 1], F32, tag="bvar", name="bvar")
        nc.vector.tensor_scalar(bvar[:], m2[:], -1.0, LN_EPS, MULT, ADD)
        st = small.tile([128, 1], F32, tag="st", name="st")
        nc.scalar.activation(st[:], sq[:], AF.Sqrt, bias=bvar[:], scale=1.0 / D)
        rstd = small.tile([128, 1], F32, tag="rstd", name="rstd")
        nc.vector.reciprocal(rstd[:], st[:])
        nmr = small.tile([128, 1], F32, tag="nmr", name="nmr")
        nc.vector.tensor_tensor(nmr[:], negmu[:], rstd[:], MULT)
        nc.vector.tensor_scalar(out_ap, x_ap, rstd[:], nmr[:], MULT, ADD)

    with tile.TileContext(nc) as tc:
        # Pool lifetimes form two LIFO stacks (SBUF left/right).
        glob = tc.alloc_tile_pool(name="glob", bufs=1, side="right")
        small = tc.alloc_tile_pool(name="small", bufs=8, side="right")
        de = tc.alloc_tile_pool(name="de", bufs=1, side="right")  # x2 (D..end)

        ident = glob.tile([128, 128], BF16, name="ident")
        make_identity(nc, ident[:])
        dump = glob.tile([128, D], BF16, name="dump")
        x2_sb = de.tile([128, 4, D], F32, name="x2_sb")

        # bc: everything attention-scoped (released at end of attention)
        bc = tc.alloc_tile_pool(name="bc", bufs=1, side="right")
        hT = bc.tile([128, 8, ROWS], BF16, name="hT")
        QT = bc.tile([128, 8, CHUNK], BF16, name="QT")     # [hd-pair, pair, qi]
        KT = bc.tile([128, 8, ROWS], BF16, name="KT")
        Vp = bc.tile([128, 6, 16 * 65], PV_DT, name="Vp")  # per-head 65-col groups
        amask_t = bc.tile([128, H, 384], BF16, name="amask_t")
        valid_t = bc.tile([128, 6], F32, name="valid_t")

        # ---------------- Phase A: LN1 + h^T ----------------
        # xl blocks first: Q only needs hT rows 256:768 (blocks 2-5), so all
        # Q chains can run while the halo blocks' LN is still in flight.
        _scA = nc.enter_named_scope('A', False)[0]
        xlp = tc.alloc_tile_pool(name="xlp", bufs=1, side="left")  # x local, A..D
        pa = tc.alloc_tile_pool(name="pa", bufs=2, side="left")
        psab = tc.alloc_tile_pool(name="psab", bufs=2, space="PSUM")
        xl_sb = xlp.tile([128, 4, D], BF16, name="xl_sb")
        xl_r = xl_d.rearrange("(q p) d -> p q d", p=128)
        for q in range(4):
            nc.sync.dma_start(xl_sb[:, q], xl_r[:, q])
        xh_sb = pa.tile([128, 2, D], BF16, name="xh_sb", bufs=1)
        xh_r = xh_d.rearrange("(q p) d -> p q d", p=128)
        nc.sync.dma_start(xh_sb[:, 0], xh_r[:, 0])
        nc.sync.dma_start(xh_sb[:, 1], xh_r[:, 1])
        nc.sync.dma_start(valid_t[:], valid_d.rearrange("k p -> p k"))
        # amask is only needed by the softmax; keep it off the SP DGE queue so
        # the first weight chunks aren't stuck behind its 1.5 MB.
        nc.scalar.dma_start(amask_t[:], amask_d)
        if has_qk_bias:
            qkb_t = bc.tile([128, 2, 8], F32, name="qkb_t")
            nc.scalar.dma_start(qkb_t[:], qkb_d.rearrange("t g p -> p t g"))

        wb = tc.alloc_tile_pool(name="wb", bufs=2, side="right")
        wqs = []
        for g in range(2):
            wq = wb.tile([128, 8, 512], BF16, tag="wq", name="wq", bufs=2)
            nc.sync.dma_start(
                wq[:], wqkv_d[:, g * 512:(g + 1) * 512]
                .rearrange("(ko p) n -> p ko n", p=128))
            wqs.append(wq)
        wv = wb.tile([128, 8, D], BF16, tag="wv", name="wv", bufs=1)
        nc.sync.dma_start(wv[:],
                          wqkv_d[:, 2 * D:3 * D].rearrange("(ko p) n -> p ko n", p=128))

        def a_block(blk):
            x_ap = xh_sb[:, blk] if blk < 2 else xl_sb[:, blk - 2]
            h_pre = pa.tile([128, D], BF16, tag="h_pre", name="h_pre")
            ln_block(tc, x_ap, h_pre[:], small, dump)
            for kc in range(8):
                pst = psab.tile([128, 128], BF16, tag="tr", name="ps_tr")
                nc.tensor.transpose(pst[:], h_pre[:, kc * 128:(kc + 1) * 128],
                                    ident[:])
                if kc % 2 == 0:
                    nc.vector.tensor_copy(hT[:, kc, blk * 128:(blk + 1) * 128], pst[:])
                else:
                    nc.scalar.copy(hT[:, kc, blk * 128:(blk + 1) * 128], pst[:])

        for blk in (2, 3, 4, 5):
            a_block(blk)

        nc.leave_named_scope('A', _scA, False)
        # ---------------- Phase B: Q (all pairs), halo LN, V ----------------
        _scB = nc.enter_named_scope('B', False)[0]
        for g in range(2):
            for pp in range(4):
                hp = g * 4 + pp
                psq = psab.tile([128, CHUNK], F32, tag="q", name="ps_q", bufs=1)
                for ko in range(8):
                    nc.tensor.matmul(psq[:], wqs[g][:, ko, pp * 128:(pp + 1) * 128],
                                     hT[:, ko, HALO:ROWS],
                                     start=(ko == 0), stop=(ko == 7))
                if has_qk_bias:
                    nc.scalar.activation(QT[:, hp], psq[:], AF.Identity,
                                         bias=qkb_t[:, 0, hp:hp + 1])
                else:
                    nc.scalar.copy(QT[:, hp], psq[:])

        # halo LN runs on DVE/ACT underneath the Q matmuls
        for blk in (0, 1):
            a_block(blk)
        pa.release()

        # V: full 768 rows (needs the halo blocks)
        for rb in range(6):
            vp_rb = Vp[:, rb].rearrange("p (h c) -> p h c", c=65)
            for nh in range(2):
                psv = psab.tile([128, 512], F32, tag="v", name="ps_v", bufs=2)
                for ko in range(8):
                    nc.tensor.matmul(psv[:],
                                     hT[:, ko, rb * 128:(rb + 1) * 128],
                                     wv[:, ko, nh * 512:(nh + 1) * 512],
                                     start=(ko == 0), stop=(ko == 7))
                # heads nh*8 .. nh*8+8 of this row-block
                nc.vector.tensor_copy(
                    vp_rb[:, nh * 8:(nh + 1) * 8, 0:64],
                    psv[:].rearrange("p (h c) -> p h c", c=64))
            nc.vector.memset(vp_rb[:, :, 64:65], 1.0)
            nc.vector.tensor_scalar_mul(Vp[:, rb], Vp[:, rb], valid_t[:, rb:rb + 1])
        psab.release()

        nc.leave_named_scope('B', _scB, False)
        _scC = nc.enter_named_scope('C', False)[0]
        cd = tc.alloc_tile_pool(name="cd", bufs=1, side="left")
        ddw = tc.alloc_tile_pool(name="ddw", bufs=1, side="left")
        O_sb = cd.tile([128, 4, D], BF16, name="O_sb")

        sp = tc.alloc_tile_pool(name="sp", bufs=2, side="right")
        psk_p = tc.alloc_tile_pool(name="psk", bufs=2, space="PSUM")
        psc = tc.alloc_tile_pool(name="psc", bufs=2, space="PSUM")
        pso = tc.alloc_tile_pool(name="pso", bufs=2, space="PSUM")

        wpj = None
        for g in range(2):
            wk = wb.tile([128, 8, 512], BF16, tag="wk", name="wk", bufs=2)
            nc.sync.dma_start(
                wk[:], wqkv_d[:, D + g * 512:D + (g + 1) * 512]
                .rearrange("(ko p) n -> p ko n", p=128))
            for pp in range(4):
                hp = g * 4 + pp
                if hp == 4:
                    # prefetch proj weights while attention still runs
                    wpj = ddw.tile([128, 8, D], BF16, name="wpj")
                    nc.sync.dma_start(wpj[:],
                                      wproj_d.rearrange("(ko p) n -> p ko n", p=128))
                # K for head-pair hp (two 384-wide chains)
                for n0 in (0, 384):
                    psk = psk_p.tile([128, 384], F32, tag="k", name="ps_k")
                    for ko in range(8):
                        nc.tensor.matmul(psk[:],
                                         wk[:, ko, pp * 128:(pp + 1) * 128],
                                         hT[:, ko, n0:n0 + 384],
                                         start=(ko == 0), stop=(ko == 7))
                    if has_qk_bias:
                        nc.scalar.activation(KT[:, hp, n0:n0 + 384], psk[:],
                                             AF.Identity,
                                             bias=qkb_t[:, 1, hp:hp + 1])
                    else:
                        nc.scalar.copy(KT[:, hp, n0:n0 + 384], psk[:])

                # attention for this head-pair
                S_pr = sp.tile([128, 2, 1536], F32, tag="S", name="S_pr")
                P_pr = sp.tile([128, 2, 1536], PV_DT, tag="P", name="P_pr")
                for kb in range(6):
                    qlo, qhi, il = _kb_span(kb)
                    w = qhi - qlo
                    # [128, 2, 512]: head-halves on PSUM bank boundaries
                    pss = psc.tile([128, 2, 512], F32, tag="s", name="ps_s")
                    for hh in range(2):
                        pb = hh * 64
                        nc.tensor.matmul(pss[:, hh, :w],
                                         KT[pb:pb + 64, hp, kb * 128:(kb + 1) * 128],
                                         QT[pb:pb + 64, hp, qlo:qhi],
                                         start=True, stop=True)
                    nc.vector.tensor_tensor(
                        S_pr[:, :, _KB_OFF[kb]:_KB_OFF[kb] + w],
                        pss[:, :, 0:w],
                        amask_t[:, 2 * hp:2 * hp + 2, il:il + w], ADD)
                # split per head-half so PV of hh=0 starts while hh=1 exps
                nc.scalar.activation(P_pr[:, 0], S_pr[:, 0], AF.Exp)
                nc.scalar.activation(P_pr[:, 1], S_pr[:, 1], AF.Exp)
                for hh in range(2):
                    h_i = hp * 2 + hh
                    # four query-blocks share one PSUM bank: one reciprocal
                    po = pso.tile([128, 4, 65], F32, tag="o", name="ps_o")
                    for qb in range(4):
                        for t in range(3):
                            kb = qb + t
                            qlo, _, _ = _kb_span(kb)
                            pcol = _KB_OFF[kb] + qb * 128 - qlo
                            nc.tensor.matmul(po[:, qb], P_pr[:, hh, pcol:pcol + 128],
                                             Vp[:, kb, h_i * 65:(h_i + 1) * 65],
                                             start=(t == 0), stop=(t == 2))
                    rec = small.tile([128, 4], F32, tag="rec", name="rec")
                    nc.vector.reciprocal(rec[:], po[:, :, 64])
                    for qb in range(4):
                        if qb % 2 == 0:
                            # balance the divide work across ACT and DVE
                            nc.scalar.activation(
                                O_sb[:, qb, h_i * 64:(h_i + 1) * 64],
                                po[:, qb, 0:64], AF.Copy, scale=rec[:, qb:qb + 1])
                        else:
                            nc.vector.tensor_scalar_mul(
                                O_sb[:, qb, h_i * 64:(h_i + 1) * 64],
                                po[:, qb, 0:64], rec[:, qb:qb + 1])
        pso.release()
        psc.release()
        psk_p.release()
        sp.release()
        wb.release()
        bc.release()  # frees hT/QT/KT/Vp/amask

        nc.leave_named_scope('C', _scC, False)
        # ---------------- Phase F pools (right side) ------------------------
        ff = tc.alloc_tile_pool(name="ff", bufs=1, side="right")
        ffT = ff.tile([128, 32, CHUNK], BF16, name="ffT")
        y_sb = ff.tile([128, 4, D], F32, name="y_sb")
        h2T = ff.tile([128, 8, CHUNK], BF16, name="h2T")
        # all of wfc2 stays resident (64 KB/partition in bf16): its DMA runs
        # through D/F1 and fc2 never waits on weights.
        w2all = ff.tile([128, 32, D], BF16, name="w2all")
        w2_r = wfc2_d.rearrange("(fo p) n -> p fo n", p=128)
        if has_fc1_bias:
            b1_t = ff.tile([128, 32], F32, name="b1_t")
            nc.sync.dma_start(b1_t[:], b1_d.rearrange("(fo p) -> p fo", p=128))
        wf = tc.alloc_tile_pool(name="wf", bufs=2, side="right")
        for g in range(2):
            nc.sync.dma_start(w2all[:, g * 2:(g + 1) * 2], w2_r[:, g * 2:(g + 1) * 2])

        # ------- Phase D: O^T + proj + residual + LN2 + h2^T, per qc --------
        _scD = nc.enter_named_scope('D', False)[0]
        dd = tc.alloc_tile_pool(name="dd", bufs=1, side="left")
        OT = dd.tile([128, 8, CHUNK], BF16, name="OT")
        pe_ = tc.alloc_tile_pool(name="pe", bufs=2, side="left")
        psd = tc.alloc_tile_pool(name="psd", bufs=2, space="PSUM")
        h2_pres = []
        for qc in range(4):
            for fc in range(8):
                pst = psd.tile([128, 128], BF16, tag="tr", name="ps_tr2")
                nc.tensor.transpose(pst[:], O_sb[:, qc, fc * 128:(fc + 1) * 128],
                                    ident[:])
                if fc % 2 == 0:
                    nc.vector.tensor_copy(OT[:, fc, qc * 128:(qc + 1) * 128], pst[:])
                else:
                    nc.scalar.copy(OT[:, fc, qc * 128:(qc + 1) * 128], pst[:])
            for nh in range(2):
                psp = psd.tile([128, 512], F32, tag="p", name="ps_p")
                for fc in range(8):
                    nc.tensor.matmul(psp[:], OT[:, fc, qc * 128:(qc + 1) * 128],
                                     wpj[:, fc, nh * 512:(nh + 1) * 512],
                                     start=(fc == 0), stop=(fc == 7))
                nc.vector.tensor_tensor(x2_sb[:, qc, nh * 512:(nh + 1) * 512],
                                        psp[:], xl_sb[:, qc, nh * 512:(nh + 1) * 512],
                                        ADD)
            # LN2 for this chunk runs on DVE/ACT under the next chunk's proj;
            # its PE transposes are deferred below so the in-order PE queue
            # never waits on the serial LN latency.
            h2_pre = pe_.tile([128, D], BF16, tag="h2_pre", name="h2_pre",
                              bufs=4)
            ln_block(tc, x2_sb[:, qc], h2_pre[:], small, dump)
            h2_pres.append(h2_pre)
        for qc in range(4):
            for kc in range(8):
                pst = psd.tile([128, 128], BF16, tag="tr", name="ps_tr2")
                nc.tensor.transpose(pst[:], h2_pres[qc][:, kc * 128:(kc + 1) * 128],
                                    ident[:])
                if kc % 2 == 0:
                    nc.vector.tensor_copy(h2T[:, kc, qc * 128:(qc + 1) * 128], pst[:])
                else:
                    nc.scalar.copy(h2T[:, kc, qc * 128:(qc + 1) * 128], pst[:])
        psd.release()
        pe_.release()
        dd.release()
        ddw.release()
        cd.release()
        xlp.release()

        nc.leave_named_scope('D', _scD, False)
        # ---------------- Phase F1: fc1 + GELU ----------------
        _scF1 = nc.enter_named_scope('F1', False)[0]
        psf = tc.alloc_tile_pool(name="psf", bufs=2, space="PSUM")
        for g in range(8):
            w1 = wf.tile([128, 8, 512], BF16, tag="w1", name="w1", bufs=2)
            nc.sync.dma_start(
                w1[:], wfc1_d[:, g * 512:(g + 1) * 512]
                .rearrange("(ko p) n -> p ko n", p=128))
            # trickle the rest of wfc2 between w1 chunks (4+28 = all 32)
            c = 4 + g * 3
            nw = 3 if g < 7 else 7
            nc.sync.dma_start(w2all[:, c:c + nw], w2_r[:, c:c + nw])
            for f4 in range(4):
                ffc = g * 4 + f4
                psq = psf.tile([128, 512], F32, tag="f", name="ps_f")
                for ko in range(8):
                    nc.tensor.matmul(psq[:], w1[:, ko, f4 * 128:(f4 + 1) * 128],
                                     h2T[:, ko, :], start=(ko == 0), stop=(ko == 7))
                if has_fc1_bias:
                    nc.scalar.activation(ffT[:, ffc, :], psq[:], AF.Gelu,
                                         bias=b1_t[:, ffc:ffc + 1])
                else:
                    nc.scalar.activation(ffT[:, ffc, :], psq[:], AF.Gelu)
        psf.release()

        nc.leave_named_scope('F1', _scF1, False)
        # ---------------- Phase F2: fc2 + residual + store ----------------
        _scF2 = nc.enter_named_scope('F2', False)[0]
        psy = tc.alloc_tile_pool(name="psy", bufs=4, space="PSUM")
        y_dr = y_d.rearrange("(q p) d -> p q d", p=128)
        # qc-outer: each 128-row output chunk finishes its accumulation a
        # quarter of the way in, so its residual add and store overlap the
        # remaining compute instead of piling up at the end.
        for qc in range(4):
            ys = [psy.tile([128, 512], F32, tag="y", name=f"ps_y{nh}")
                  for nh in range(2)]
            for ffc in range(32):
                for nh in range(2):
                    nc.tensor.matmul(ys[nh][:],
                                     ffT[:, ffc, qc * 128:(qc + 1) * 128],
                                     w2all[:, ffc, nh * 512:(nh + 1) * 512],
                                     start=(ffc == 0), stop=(ffc == 31))
            for nh in range(2):
                nc.vector.tensor_tensor(y_sb[:, qc, nh * 512:(nh + 1) * 512],
                                        ys[nh][:],
                                        x2_sb[:, qc, nh * 512:(nh + 1) * 512], ADD)
            nc.sync.dma_start(y_dr[:, qc], y_sb[:, qc])
        psy.release()
        wf.release()
        ff.release()
        de.release()
        small.release()
        glob.release()

        nc.leave_named_scope('F2', _scF2, False)

    nc.compile()
    return nc


def kernel(x, qkv_w, qkv_b, proj_w, proj_b, ln1_g, ln1_b, ln2_g, ln2_b,
           fc1_w, fc1_b, fc2_w, fc2_b):
    from concourse.bass_utils import run_bass_kernel_spmd

    x = np.ascontiguousarray(np.asarray(x, dtype=np.float32))
    f32 = lambda a: np.asarray(a, dtype=np.float32)
    qkv_w, qkv_b = f32(qkv_w), f32(qkv_b)
    proj_w, proj_b = f32(proj_w), f32(proj_b)
    fc1_w, fc1_b = f32(fc1_w), f32(fc1_b)
    fc2_w, fc2_b = f32(fc2_w), f32(fc2_b)
    ln1_g, ln1_b = f32(ln1_g), f32(ln1_b)
    ln2_g, ln2_b = f32(ln2_g), f32(ln2_b)

    # Host-side folding: LN affine into the following weight/bias; HD^-0.5 into Wk.
    import ml_dtypes
    bf = ml_dtypes.bfloat16
    scale = HD ** -0.5
    wqkv = ln1_g[:, None] * qkv_w
    bqkv = qkv_b + ln1_b @ qkv_w
    wqkv = np.ascontiguousarray(wqkv)
    wqkv[:, D:2 * D] *= scale
    bqkv = bqkv.copy()
    bqkv[D:2 * D] *= scale
    wfc1 = np.ascontiguousarray(ln2_g[:, None] * fc1_w)
    bfc1 = fc1_b + ln2_b @ fc1_w
    wqkv = np.ascontiguousarray(wqkv.astype(bf))
    wproj16 = np.ascontiguousarray(proj_w.astype(bf))
    wfc1 = np.ascontiguousarray(wfc1.astype(bf))
    wfc216 = np.ascontiguousarray(fc2_w.astype(bf))

    if np.any(bqkv[2 * D:]) or np.any(proj_b) or np.any(fc2_b):
        raise NotImplementedError("nonzero v/proj/fc2 bias not supported")

    has_qk_bias = bool(np.any(bqkv[:2 * D]))
    has_fc1_bias = bool(np.any(bfc1))
    key = (has_qk_bias, has_fc1_bias)
    if key not in _cache:
        _cache[key] = _build_program(*key)
    nc = _cache[key]

    amask = _build_amask()
    in_maps = []
    for c in range(NCORES):
        b, ck = c // 4, c % 4
        g0 = ck * CHUNK
        xl = np.ascontiguousarray(x[b, g0:g0 + CHUNK].astype(bf))
        if ck > 0:
            xhalo = np.ascontiguousarray(x[b, g0 - HALO:g0].astype(bf))
        else:
            xhalo = np.zeros((HALO, D), bf)
        valid = np.ones((6, 128), np.float32)
        if ck == 0:
            valid[:2] = 0.0
        m = {"xh": xhalo, "xl": xl, "wqkv": wqkv, "wproj": wproj16,
             "wfc1": wfc1, "wfc2": wfc216, "amask": amask, "valid": valid}
        if has_qk_bias:
            m["qkbias"] = np.ascontiguousarray(
                bqkv[:2 * D].reshape(2, 8, 128))
        if has_fc1_bias:
            m["b1"] = bfc1
        in_maps.append(m)

    res = run_bass_kernel_spmd(nc, in_maps, core_ids=list(range(NCORES)))
    y = np.empty((B, N, D), np.float32)
    for c in range(NCORES):
        b, ck = c // 4, c % 4
        y[b, ck * CHUNK:(ck + 1) * CHUNK] = res.results[c]["y"]
    return y


# revision 60
# speedup vs baseline: 1.1582x; 1.0018x over previous
"""Trainium2 Bass kernel for nn_AdvancedTransformerBlock_15006615733156.

Pre-norm transformer block: LN1 -> QKV -> sliding-window causal attention with
ALiBi (window 256) -> proj residual -> LN2 -> FFN (exact GELU) residual.
B=2, N=2048, D=1024, H=16, HD=64.

Sharding: 8 cores = batch(2) x sequence(4 chunks of 512 rows). The 256-wide
sliding window means each core only needs a 256-row halo of x before its
chunk — no collectives. Chunk-0 cores get a zeroed halo plus a `valid` mask
that zeroes halo V' rows (kills both numerator and softmax denominator).

On-chip layout: scores are computed transposed (S_t[kj, qi]) so the
probability tile is directly consumable as matmul lhsT for PV; the softmax
denominator comes from an appended ones-column in V'. All big matmuls run in
bf16 (fast-weight-load on LDWEIGHTS, low PE power); accumulation stays fp32
in PSUM, and LN stats / residual adds stay fp32.

Phase structure is chosen to keep the Tensor engine densely fed: the HW
activity manager halves the PE duty limit (k=4/8) within ~10us of the PE
going idle, so PE-sparse phases run their matmuls at half rate. QKV
projections are therefore interleaved per-head-pair with that head-pair's
attention, and LN2 is folded into the proj/residual loop.
"""
import sys, math, os
sys.path.insert(0, '/opt/trn_rl_repo')
import numpy as np

B, N, D, H, HD, WIN = 2, 2048, 1024, 16, 64, 256
CHUNK, HALO, ROWS = 512, 256, 768
NEG = -1e30
LN_EPS = 1e-5
NCORES = 8

_cache = {}


def _alibi_slopes(n):
    closest = 2 ** math.floor(math.log2(n))
    base = 2.0 ** (-(2.0 ** (-(math.log2(closest) - 3))))
    return np.power(base, np.arange(1, closest + 1)).astype(np.float32)


def _build_amask():
    """Additive pre-softmax bias, [128, H, 384] bf16 (partition-major so the
    DMA moves one 12 KB contiguous run per partition).

    Softmax over keys j is invariant to any per-query-column constant, so the
    reference's "+1 inside window" and the -slope*i part of the ALiBi term
    drop out; what remains is slope*(j - i) <= 0 inside the band, -1e30
    outside. Values near each column's max are near zero, so bf16's relative
    rounding cannot disturb the softmax weights meaningfully.
    """
    import ml_dtypes
    sl = _alibi_slopes(H)
    j = np.arange(128)[:, None]
    i = np.arange(384)[None, :]
    band = ((i - j) >= 0) & ((i - j) <= 255)
    out = np.where(band[None], sl[:, None, None] * (j - i)[None], NEG)
    return np.ascontiguousarray(
        out.astype(ml_dtypes.bfloat16).transpose(1, 0, 2))


def _kb_span(kb):
    qlo = max(0, kb * 128 - 256)
    qhi = min(512, kb * 128 + 128)
    return qlo, qhi, qlo - (kb * 128 - 256)


_KB_OFF = [0]
for _kb in range(6):
    _q0, _q1, _ = _kb_span(_kb)
    _KB_OFF.append(_KB_OFF[-1] + (_q1 - _q0))  # offsets into the 1536-wide S row


def _build_program(has_qk_bias, has_fc1_bias):
    import concourse.bass as bass
    import concourse.tile as tile
    from concourse import bacc, mybir
    from concourse.masks import make_identity

    F32 = mybir.dt.float32
    BF16 = mybir.dt.bfloat16
    # 16-bit P/V' runs PV at 1 cyc/row and gets FWL on the weight load; fp16's
    # 10-bit mantissa keeps softmax-prob rounding at ~5e-4.
    _pv = os.environ.get("K_PV_DT", "f16")
    PV_DT = {"f32": F32, "bf16": BF16, "f16": mybir.dt.float16}[_pv]
    AF = mybir.ActivationFunctionType
    ADD, MULT = mybir.AluOpType.add, mybir.AluOpType.mult

    nc = bacc.Bacc("TRN2", target_bir_lowering=False, debug=False,
                   num_devices=NCORES)

    xh_d = nc.dram_tensor("xh", [HALO, D], BF16, kind="ExternalInput").ap()
    xl_d = nc.dram_tensor("xl", [CHUNK, D], BF16, kind="ExternalInput").ap()
    wqkv_d = nc.dram_tensor("wqkv", [D, 3 * D], BF16, kind="ExternalInput").ap()
    wproj_d = nc.dram_tensor("wproj", [D, D], BF16, kind="ExternalInput").ap()
    wfc1_d = nc.dram_tensor("wfc1", [D, 4 * D], BF16, kind="ExternalInput").ap()
    wfc2_d = nc.dram_tensor("wfc2", [4 * D, D], BF16, kind="ExternalInput").ap()
    amask_d = nc.dram_tensor("amask", [128, H, 384], BF16, kind="ExternalInput").ap()
    valid_d = nc.dram_tensor("valid", [6, 128], F32, kind="ExternalInput").ap()
    if has_qk_bias:
        qkb_d = nc.dram_tensor("qkbias", [2, 8, 128], F32, kind="ExternalInput").ap()
    if has_fc1_bias:
        b1_d = nc.dram_tensor("b1", [4 * D], F32, kind="ExternalInput").ap()
    y_d = nc.dram_tensor("y", [CHUNK, D], F32, kind="ExternalOutput").ap()

    def ln_block(tc, x_ap, out_ap, small, dump):
        """LayerNorm (no affine) of [128, 1024]: out = (x - mu) * rstd.

        Stats split across engines in parallel: ACT computes E[x^2] via
        Square+accum while DVE reduces E[x]; var = E[x^2] - mu^2.
        """
        sq = small.tile([128, 1], F32, tag="sq", name="sq")
        nc.scalar.activation(dump[:], x_ap, AF.Square, accum_out=sq[:])
        sums = small.tile([128, 1], F32, tag="sums", name="sums")
        nc.vector.tensor_reduce(sums[:], x_ap, mybir.AxisListType.X,
                                mybir.AluOpType.add)
        negmu = small.tile([128, 1], F32, tag="negmu", name="negmu")
        nc.vector.tensor_scalar_mul(negmu[:], sums[:], -1.0 / D)
        m2 = small.tile([128, 1], F32, tag="m2", name="m2")
        nc.vector.tensor_tensor(m2[:], negmu[:], negmu[:], MULT)
        bvar = small.tile([128, 1], F32, tag="bvar", name="bvar")
        nc.vector.tensor_scalar(bvar[:], m2[:], -1.0, LN_EPS, MULT, ADD)
        st = small.tile([128, 1], F32, tag="st", name="st")
        nc.scalar.activation(st[:], sq[:], AF.Sqrt, bias=bvar[:], scale=1.0 / D)
        rstd = small.tile([128, 1], F32, tag="rstd", name="rstd")
        nc.vector.reciprocal(rstd[:], st[:])
        nmr = small.tile([128, 1], F32, tag="nmr", name="nmr")
        nc.vector.tensor_tensor(nmr[:], negmu[:], rstd[:], MULT)
        nc.vector.tensor_scalar(out_ap, x_ap, rstd[:], nmr[:], MULT, ADD)

    with tile.TileContext(nc) as tc:
        # Pool lifetimes form two LIFO stacks (SBUF left/right).
        glob = tc.alloc_tile_pool(name="glob", bufs=1, side="right")
        small = tc.alloc_tile_pool(name="small", bufs=8, side="right")
        de = tc.alloc_tile_pool(name="de", bufs=1, side="right")  # x2 (D..end)

        ident = glob.tile([128, 128], BF16, name="ident")
        make_identity(nc, ident[:])
        dump = glob.tile([128, D], BF16, name="dump")
        x2_sb = de.tile([128, 4, D], F32, name="x2_sb")

        # bc: everything attention-scoped (released at end of attention)
        bc = tc.alloc_tile_pool(name="bc", bufs=1, side="right")
        hT = bc.tile([128, 8, ROWS], BF16, name="hT")
        QT = bc.tile([128, 8, CHUNK], BF16, name="QT")     # [hd-pair, pair, qi]
        KT = bc.tile([128, 8, ROWS], BF16, name="KT")
        Vp = bc.tile([128, 6, 16 * 65], PV_DT, name="Vp")  # per-head 65-col groups
        amask_t = bc.tile([128, H, 384], BF16, name="amask_t")
        valid_t = bc.tile([128, 6], F32, name="valid_t")

        # ---------------- Phase A: LN1 + h^T ----------------
        # xl blocks first: Q only needs hT rows 256:768 (blocks 2-5), so all
        # Q chains can run while the halo blocks' LN is still in flight.
        _scA = nc.enter_named_scope('A', False)[0]
        xlp = tc.alloc_tile_pool(name="xlp", bufs=1, side="left")  # x local, A..D
        pa = tc.alloc_tile_pool(name="pa", bufs=2, side="left")
        psab = tc.alloc_tile_pool(name="psab", bufs=2, space="PSUM")
        xl_sb = xlp.tile([128, 4, D], BF16, name="xl_sb")
        xl_r = xl_d.rearrange("(q p) d -> p q d", p=128)
        for q in range(4):
            nc.sync.dma_start(xl_sb[:, q], xl_r[:, q])
        xh_sb = pa.tile([128, 2, D], BF16, name="xh_sb", bufs=1)
        xh_r = xh_d.rearrange("(q p) d -> p q d", p=128)
        nc.sync.dma_start(xh_sb[:, 0], xh_r[:, 0])
        nc.sync.dma_start(xh_sb[:, 1], xh_r[:, 1])
        nc.sync.dma_start(valid_t[:], valid_d.rearrange("k p -> p k"))
        # amask is only needed by the softmax; keep it off the SP DGE queue so
        # the first weight chunks aren't stuck behind its 1.5 MB.
        nc.scalar.dma_start(amask_t[:], amask_d)
        if has_qk_bias:
            qkb_t = bc.tile([128, 2, 8], F32, name="qkb_t")
            nc.scalar.dma_start(qkb_t[:], qkb_d.rearrange("t g p -> p t g"))

        wb = tc.alloc_tile_pool(name="wb", bufs=2, side="right")
        wqs, wks = [], []
        for g in range(2):
            wq = wb.tile([128, 8, 512], BF16, tag="wq", name="wq", bufs=2)
            nc.sync.dma_start(
                wq[:], wqkv_d[:, g * 512:(g + 1) * 512]
                .rearrange("(ko p) n -> p ko n", p=128))
            wqs.append(wq)
        wv = wb.tile([128, 8, D], BF16, tag="wv", name="wv", bufs=1)
        nc.sync.dma_start(wv[:],
                          wqkv_d[:, 2 * D:3 * D].rearrange("(ko p) n -> p ko n", p=128))
        # K weights issued now too: they arrive under Q/V compute, so the
        # first K chain never waits at the attention-loop entry.
        for g in range(2):
            wk = wb.tile([128, 8, 512], BF16, tag="wk", name="wk", bufs=2)
            nc.sync.dma_start(
                wk[:], wqkv_d[:, D + g * 512:D + (g + 1) * 512]
                .rearrange("(ko p) n -> p ko n", p=128))
            wks.append(wk)

        def a_block(blk):
            x_ap = xh_sb[:, blk] if blk < 2 else xl_sb[:, blk - 2]
            h_pre = pa.tile([128, D], BF16, tag="h_pre", name="h_pre")
            ln_block(tc, x_ap, h_pre[:], small, dump)
            for kc in range(8):
                pst = psab.tile([128, 128], BF16, tag="tr", name="ps_tr")
                nc.tensor.transpose(pst[:], h_pre[:, kc * 128:(kc + 1) * 128],
                                    ident[:])
                if kc % 2 == 0:
                    nc.vector.tensor_copy(hT[:, kc, blk * 128:(blk + 1) * 128], pst[:])
                else:
                    nc.scalar.copy(hT[:, kc, blk * 128:(blk + 1) * 128], pst[:])

        for blk in (2, 3, 4, 5):
            a_block(blk)

        nc.leave_named_scope('A', _scA, False)
        # ---------------- Phase B: Q (all pairs), halo LN, V ----------------
        _scB = nc.enter_named_scope('B', False)[0]
        for g in range(2):
            for pp in range(4):
                hp = g * 4 + pp
                psq = psab.tile([128, CHUNK], F32, tag="q", name="ps_q", bufs=1)
                for ko in range(8):
                    nc.tensor.matmul(psq[:], wqs[g][:, ko, pp * 128:(pp + 1) * 128],
                                     hT[:, ko, HALO:ROWS],
                                     start=(ko == 0), stop=(ko == 7))
                if has_qk_bias:
                    nc.scalar.activation(QT[:, hp], psq[:], AF.Identity,
                                         bias=qkb_t[:, 0, hp:hp + 1])
                else:
                    nc.scalar.copy(QT[:, hp], psq[:])

        # halo LN runs on DVE/ACT underneath the Q matmuls
        for blk in (0, 1):
            a_block(blk)
        pa.release()

        # V: full 768 rows (needs the halo blocks)
        for rb in range(6):
            vp_rb = Vp[:, rb].rearrange("p (h c) -> p h c", c=65)
            for nh in range(2):
                psv = psab.tile([128, 512], F32, tag="v", name="ps_v", bufs=2)
                for ko in range(8):
                    nc.tensor.matmul(psv[:],
                                     hT[:, ko, rb * 128:(rb + 1) * 128],
                                     wv[:, ko, nh * 512:(nh + 1) * 512],
                                     start=(ko == 0), stop=(ko == 7))
                # heads nh*8 .. nh*8+8 of this row-block
                nc.vector.tensor_copy(
                    vp_rb[:, nh * 8:(nh + 1) * 8, 0:64],
                    psv[:].rearrange("p (h c) -> p h c", c=64))
            nc.vector.memset(vp_rb[:, :, 64:65], 1.0)
            nc.vector.tensor_scalar_mul(Vp[:, rb], Vp[:, rb], valid_t[:, rb:rb + 1])
        psab.release()

        nc.leave_named_scope('B', _scB, False)
        _scC = nc.enter_named_scope('C', False)[0]
        cd = tc.alloc_tile_pool(name="cd", bufs=1, side="left")
        ddw = tc.alloc_tile_pool(name="ddw", bufs=1, side="left")
        O_sb = cd.tile([128, 4, D], BF16, name="O_sb")

        sp = tc.alloc_tile_pool(name="sp", bufs=2, side="right")
        psk_p = tc.alloc_tile_pool(name="psk", bufs=2, space="PSUM")
        psc = tc.alloc_tile_pool(name="psc", bufs=2, space="PSUM")
        pso = tc.alloc_tile_pool(name="pso", bufs=2, space="PSUM")

        wpj = None
        for g in range(2):
            wk = wks[g]
            for pp in range(4):
                hp = g * 4 + pp
                if hp == 4:
                    # prefetch proj weights while attention still runs
                    wpj = ddw.tile([128, 8, D], BF16, name="wpj")
                    nc.sync.dma_start(wpj[:],
                                      wproj_d.rearrange("(ko p) n -> p ko n", p=128))
                # K for head-pair hp (two 384-wide chains)
                for n0 in (0, 384):
                    psk = psk_p.tile([128, 384], F32, tag="k", name="ps_k")
                    for ko in range(8):
                        nc.tensor.matmul(psk[:],
                                         wk[:, ko, pp * 128:(pp + 1) * 128],
                                         hT[:, ko, n0:n0 + 384],
                                         start=(ko == 0), stop=(ko == 7))
                    if has_qk_bias:
                        nc.scalar.activation(KT[:, hp, n0:n0 + 384], psk[:],
                                             AF.Identity,
                                             bias=qkb_t[:, 1, hp:hp + 1])
                    else:
                        nc.scalar.copy(KT[:, hp, n0:n0 + 384], psk[:])

                # attention for this head-pair
                S_pr = sp.tile([128, 2, 1536], F32, tag="S", name="S_pr")
                P_pr = sp.tile([128, 2, 1536], PV_DT, tag="P", name="P_pr")
                for kb in range(6):
                    qlo, qhi, il = _kb_span(kb)
                    w = qhi - qlo
                    # [128, 2, 512]: head-halves on PSUM bank boundaries
                    pss = psc.tile([128, 2, 512], F32, tag="s", name="ps_s")
                    for hh in range(2):
                        pb = hh * 64
                        nc.tensor.matmul(pss[:, hh, :w],
                                         KT[pb:pb + 64, hp, kb * 128:(kb + 1) * 128],
                                         QT[pb:pb + 64, hp, qlo:qhi],
                                         start=True, stop=True)
                    nc.vector.tensor_tensor(
                        S_pr[:, :, _KB_OFF[kb]:_KB_OFF[kb] + w],
                        pss[:, :, 0:w],
                        amask_t[:, 2 * hp:2 * hp + 2, il:il + w], ADD)
                # split per head-half so PV of hh=0 starts while hh=1 exps
                nc.scalar.activation(P_pr[:, 0], S_pr[:, 0], AF.Exp)
                nc.scalar.activation(P_pr[:, 1], S_pr[:, 1], AF.Exp)
                for hh in range(2):
                    h_i = hp * 2 + hh
                    # four query-blocks share one PSUM bank: one reciprocal
                    po = pso.tile([128, 4, 65], F32, tag="o", name="ps_o")
                    for qb in range(4):
                        for t in range(3):
                            kb = qb + t
                            qlo, _, _ = _kb_span(kb)
                            pcol = _KB_OFF[kb] + qb * 128 - qlo
                            nc.tensor.matmul(po[:, qb], P_pr[:, hh, pcol:pcol + 128],
                                             Vp[:, kb, h_i * 65:(h_i + 1) * 65],
                                             start=(t == 0), stop=(t == 2))
                    rec = small.tile([128, 4], F32, tag="rec", name="rec")
                    nc.vector.reciprocal(rec[:], po[:, :, 64])
                    for qb in range(4):
                        if qb % 2 == 0:
                            # balance the divide work across ACT and DVE
                            nc.scalar.activation(
                                O_sb[:, qb, h_i * 64:(h_i + 1) * 64],
                                po[:, qb, 0:64], AF.Copy, scale=rec[:, qb:qb + 1])
                        else:
                            nc.vector.tensor_scalar_mul(
                                O_sb[:, qb, h_i * 64:(h_i + 1) * 64],
                                po[:, qb, 0:64], rec[:, qb:qb + 1])
        pso.release()
        psc.release()
        psk_p.release()
        sp.release()
        wb.release()
        bc.release()  # frees hT/QT/KT/Vp/amask

        nc.leave_named_scope('C', _scC, False)
        # ---------------- Phase F pools (right side) ------------------------
        ff = tc.alloc_tile_pool(name="ff", bufs=1, side="right")
        ffT = ff.tile([128, 32, CHUNK], BF16, name="ffT")
        y_sb = ff.tile([128, 4, D], F32, name="y_sb")
        h2T = ff.tile([128, 8, CHUNK], BF16, name="h2T")
        # all of wfc2 stays resident (64 KB/partition in bf16): its DMA runs
        # through D/F1 and fc2 never waits on weights.
        w2all = ff.tile([128, 32, D], BF16, name="w2all")
        w2_r = wfc2_d.rearrange("(fo p) n -> p fo n", p=128)
        if has_fc1_bias:
            b1_t = ff.tile([128, 32], F32, name="b1_t")
            nc.sync.dma_start(b1_t[:], b1_d.rearrange("(fo p) -> p fo", p=128))
        wf = tc.alloc_tile_pool(name="wf", bufs=2, side="right")
        for g in range(2):
            nc.sync.dma_start(w2all[:, g * 2:(g + 1) * 2], w2_r[:, g * 2:(g + 1) * 2])

        # ------- Phase D: O^T + proj + residual + LN2 + h2^T, per qc --------
        _scD = nc.enter_named_scope('D', False)[0]
        dd = tc.alloc_tile_pool(name="dd", bufs=1, side="left")
        OT = dd.tile([128, 8, CHUNK], BF16, name="OT")
        pe_ = tc.alloc_tile_pool(name="pe", bufs=2, side="left")
        psd = tc.alloc_tile_pool(name="psd", bufs=2, space="PSUM")
        h2_pres = []
        for qc in range(4):
            for fc in range(8):
                pst = psd.tile([128, 128], BF16, tag="tr", name="ps_tr2")
                nc.tensor.transpose(pst[:], O_sb[:, qc, fc * 128:(fc + 1) * 128],
                                    ident[:])
                if fc % 2 == 0:
                    nc.vector.tensor_copy(OT[:, fc, qc * 128:(qc + 1) * 128], pst[:])
                else:
                    nc.scalar.copy(OT[:, fc, qc * 128:(qc + 1) * 128], pst[:])
            for nh in range(2):
                psp = psd.tile([128, 512], F32, tag="p", name="ps_p")
                for fc in range(8):
                    nc.tensor.matmul(psp[:], OT[:, fc, qc * 128:(qc + 1) * 128],
                                     wpj[:, fc, nh * 512:(nh + 1) * 512],
                                     start=(fc == 0), stop=(fc == 7))
                nc.vector.tensor_tensor(x2_sb[:, qc, nh * 512:(nh + 1) * 512],
                                        psp[:], xl_sb[:, qc, nh * 512:(nh + 1) * 512],
                                        ADD)
            # LN2 for this chunk runs on DVE/ACT under the next chunk's proj;
            # its PE transposes are deferred below so the in-order PE queue
            # never waits on the serial LN latency.
            h2_pre = pe_.tile([128, D], BF16, tag="h2_pre", name="h2_pre",
                              bufs=4)
            ln_block(tc, x2_sb[:, qc], h2_pre[:], small, dump)
            h2_pres.append(h2_pre)
        for qc in range(4):
            for kc in range(8):
                pst = psd.tile([128, 128], BF16, tag="tr", name="ps_tr2")
                nc.tensor.transpose(pst[:], h2_pres[qc][:, kc * 128:(kc + 1) * 128],
                                    ident[:])
                if kc % 2 == 0:
                    nc.vector.tensor_copy(h2T[:, kc, qc * 128:(qc + 1) * 128], pst[:])
                else:
                    nc.scalar.copy(h2T[:, kc, qc * 128:(qc + 1) * 128], pst[:])
        psd.release()
        pe_.release()
        dd.release()
        ddw.release()
        cd.release()
        xlp.release()

        nc.leave_named_scope('D', _scD, False)
        # ---------------- Phase F1: fc1 + GELU ----------------
        _scF1 = nc.enter_named_scope('F1', False)[0]
        psf = tc.alloc_tile_pool(name="psf", bufs=2, space="PSUM")
        for g in range(8):
            w1 = wf.tile([128, 8, 512], BF16, tag="w1", name="w1", bufs=2)
            nc.sync.dma_start(
                w1[:], wfc1_d[:, g * 512:(g + 1) * 512]
                .rearrange("(ko p) n -> p ko n", p=128))
            # trickle the rest of wfc2 between w1 chunks (4+28 = all 32)
            c = 4 + g * 3
            nw = 3 if g < 7 else 7
            nc.sync.dma_start(w2all[:, c:c + nw], w2_r[:, c:c + nw])
            for f4 in range(4):
                ffc = g * 4 + f4
                psq = psf.tile([128, 512], F32, tag="f", name="ps_f")
                for ko in range(8):
                    nc.tensor.matmul(psq[:], w1[:, ko, f4 * 128:(f4 + 1) * 128],
                                     h2T[:, ko, :], start=(ko == 0), stop=(ko == 7))
                if has_fc1_bias:
                    nc.scalar.activation(ffT[:, ffc, :], psq[:], AF.Gelu,
                                         bias=b1_t[:, ffc:ffc + 1])
                else:
                    nc.scalar.activation(ffT[:, ffc, :], psq[:], AF.Gelu)
        psf.release()

        nc.leave_named_scope('F1', _scF1, False)
        # ---------------- Phase F2: fc2 + residual + store ----------------
        _scF2 = nc.enter_named_scope('F2', False)[0]
        psy = tc.alloc_tile_pool(name="psy", bufs=4, space="PSUM")
        y_dr = y_d.rearrange("(q p) d -> p q d", p=128)
        # qc-outer: each 128-row output chunk finishes its accumulation a
        # quarter of the way in, so its residual add and store overlap the
        # remaining compute instead of piling up at the end.
        for qc in range(4):
            ys = [psy.tile([128, 512], F32, tag="y", name=f"ps_y{nh}")
                  for nh in range(2)]
            for ffc in range(32):
                for nh in range(2):
                    nc.tensor.matmul(ys[nh][:],
                                     ffT[:, ffc, qc * 128:(qc + 1) * 128],
                                     w2all[:, ffc, nh * 512:(nh + 1) * 512],
                                     start=(ffc == 0), stop=(ffc == 31))
            for nh in range(2):
                nc.vector.tensor_tensor(y_sb[:, qc, nh * 512:(nh + 1) * 512],
                                        ys[nh][:],
                                        x2_sb[:, qc, nh * 512:(nh + 1) * 512], ADD)
                # store each half as soon as its residual add lands
                nc.sync.dma_start(y_dr[:, qc, nh * 512:(nh + 1) * 512],
                                  y_sb[:, qc, nh * 512:(nh + 1) * 512])
        psy.release()
        wf.release()
        ff.release()
        de.release()
        small.release()
        glob.release()

        nc.leave_named_scope('F2', _scF2, False)

    nc.compile()
    return nc


def kernel(x, qkv_w, qkv_b, proj_w, proj_b, ln1_g, ln1_b, ln2_g, ln2_b,
           fc1_w, fc1_b, fc2_w, fc2_b):
    from concourse.bass_utils import run_bass_kernel_spmd

    x = np.ascontiguousarray(np.asarray(x, dtype=np.float32))
    f32 = lambda a: np.asarray(a, dtype=np.float32)
    qkv_w, qkv_b = f32(qkv_w), f32(qkv_b)
    proj_w, proj_b = f32(proj_w), f32(proj_b)
    fc1_w, fc1_b = f32(fc1_w), f32(fc1_b)
    fc2_w, fc2_b = f32(fc2_w), f32(fc2_b)
    ln1_g, ln1_b = f32(ln1_g), f32(ln1_b)
    ln2_g, ln2_b = f32(ln2_g), f32(ln2_b)

    # Host-side folding: LN affine into the following weight/bias; HD^-0.5 into Wk.
    import ml_dtypes
    bf = ml_dtypes.bfloat16
    scale = HD ** -0.5
    wqkv = ln1_g[:, None] * qkv_w
    bqkv = qkv_b + ln1_b @ qkv_w
    wqkv = np.ascontiguousarray(wqkv)
    wqkv[:, D:2 * D] *= scale
    bqkv = bqkv.copy()
    bqkv[D:2 * D] *= scale
    wfc1 = np.ascontiguousarray(ln2_g[:, None] * fc1_w)
    bfc1 = fc1_b + ln2_b @ fc1_w
    wqkv = np.ascontiguousarray(wqkv.astype(bf))
    wproj16 = np.ascontiguousarray(proj_w.astype(bf))
    wfc1 = np.ascontiguousarray(wfc1.astype(bf))
    wfc216 = np.ascontiguousarray(fc2_w.astype(bf))

    if np.any(bqkv[2 * D:]) or np.any(proj_b) or np.any(fc2_b):
        raise NotImplementedError("nonzero v/proj/fc2 bias not supported")

    has_qk_bias = bool(np.any(bqkv[:2 * D]))
    has_fc1_bias = bool(np.any(bfc1))
    key = (has_qk_bias, has_fc1_bias)
    if key not in _cache:
        _cache[key] = _build_program(*key)
    nc = _cache[key]

    amask = _build_amask()
    in_maps = []
    for c in range(NCORES):
        b, ck = c // 4, c % 4
        g0 = ck * CHUNK
        xl = np.ascontiguousarray(x[b, g0:g0 + CHUNK].astype(bf))
        if ck > 0:
            xhalo = np.ascontiguousarray(x[b, g0 - HALO:g0].astype(bf))
        else:
            xhalo = np.zeros((HALO, D), bf)
        valid = np.ones((6, 128), np.float32)
        if ck == 0:
            valid[:2] = 0.0
        m = {"xh": xhalo, "xl": xl, "wqkv": wqkv, "wproj": wproj16,
             "wfc1": wfc1, "wfc2": wfc216, "amask": amask, "valid": valid}
        if has_qk_bias:
            m["qkbias"] = np.ascontiguousarray(
                bqkv[:2 * D].reshape(2, 8, 128))
        if has_fc1_bias:
            m["b1"] = bfc1
        in_maps.append(m)

    res = run_bass_kernel_spmd(nc, in_maps, core_ids=list(range(NCORES)))
    y = np.empty((B, N, D), np.float32)
    for c in range(NCORES):
        b, ck = c // 4, c % 4
        y[b, ck * CHUNK:(ck + 1) * CHUNK] = res.results[c]["y"]
    return y
